# revision 1
# baseline (speedup 1.0000x reference)
"""Causal self-attention (B=2, T=2048, D=768, H=12) on 8 TRN2 NeuronCores.

Sharding: tensor-parallel over (batch, head) pairs; 3 heads per core, one
batch per 4-core group; the host sums the 4 partial outputs per batch and
adds b_out (+ the v-bias correction through w_out).

All on-device tensors are fp16 (same PE cost as bf16, 8x less rounding
noise). Per 512-token q-window, the scores->exp->PV chain runs in "rounds"
of 2-k-chunk batches across the 3 head streams; below-diagonal chunks run
first so the window's own K-side projection (fc2) and V chunks slide into
the window's early rounds, and background PE work (that projection, the
next window's Q-side projection, the previous window's output projection,
deferred normalize phases) is interleaved between rounds so the PE keeps
running while ScalarE exp latency drains. QKV bias is folded into the
PSUM->SBUF evacuations (DVE tensor_scalar / per-partition scalar add).
The softmax normalizer uses a reciprocal on DVE plus a GPSIMD
partition_broadcast (library 'proxy'); on the last window the normalize
runs per column half as the diagonal chunks finalize each half's sumexp,
overlapping the output projection. A warm-up run of free N=1 matmuls
starts the PE p-state ramp during the initial DMA wait, and the startup
DMAs are split across the SP HWDGE and Pool SWDGE queues.

Constraints learned on real silicon (CoreSim does not model them):
GPSIMD cannot access PSUM at all; tensor_tensor cannot read two PSUM
operands; PSUM matmul outputs must start at partition 0.
"""

import numpy as np

import concourse.bass as bass
import concourse.bacc as bacc
import concourse.mybir as mybir
import concourse.tile as tile
from concourse import library_config
from concourse.masks import make_upper_triangular
from concourse.bass_utils import run_bass_kernel_spmd

B, T, D, H, HD = 2, 2048, 768, 12, 64
NCORES = 8
HPC = 3            # heads per core
CPB = NCORES // B  # cores per batch = 4
CC = D // 128      # d_model chunks of 128 = 6
TW = T // 512      # token windows of 512 = 4
KC = T // 128      # k chunks of 128 = 16
SCALE = 1.0 / float(np.sqrt(HD))

F16 = mybir.dt.float16
F32 = mybir.dt.float32
F32R = mybir.dt.float32r

EXP_BATCH = 2  # k-chunks per exp call / per s-tile (PSUM tile = 2 banks)

# Schraudolph fast-exp on DVE/Pool for stream h2's below-diagonal batches in
# late windows (relieves the ScalarE bottleneck there). exp(x) ~ bf16 bitcast
# of int16(A*x + B); ~2.4% RMS approximation error on ~11% of the attention
# weights => ~8e-3 end-to-end rel err (budget 2e-2).
SCHRAU = False
SCHRAU_A = 128.0 / np.log(2.0)
SCHRAU_B = float(127 << 7) - 7.5


def build_bass():
    nc = bacc.Bacc(None, target_bir_lowering=False)

    xT = nc.dram_tensor("xT", [CC, 128, T], F16, kind="ExternalInput")
    wqk = nc.dram_tensor("wqk", [CC, 128, 3, 128], F16, kind="ExternalInput")
    bqk = nc.dram_tensor("bqk", [128, 3], F32, kind="ExternalInput")
    wv = nc.dram_tensor("wv", [CC, 128, HPC * HD], F16, kind="ExternalInput")
    wo01d = nc.dram_tensor("wo01", [128, D], F16, kind="ExternalInput")
    wo2d = nc.dram_tensor("wo2", [HD, D], F16, kind="ExternalInput")
    outT = nc.dram_tensor("outT", [D, T], F16, kind="ExternalOutput")

    with tile.TileContext(nc) as tc:
        with (
            tc.tile_pool(name="big", bufs=1) as big,
            tc.tile_pool(name="ets", bufs=6) as ets,
            tc.tile_pool(name="scr", bufs=3) as scr,
            tc.tile_pool(name="outs", bufs=4) as outs,
            tc.tile_pool(name="psS", bufs=2, space="PSUM") as psS,
            tc.tile_pool(name="psY", bufs=3, space="PSUM") as psY,
            tc.tile_pool(name="psA", bufs=1, space="PSUM") as psA,
        ):
            # ---- SBUF persistent tiles ----
            wqks = big.tile([128, CC, 3, 128], F16, tag="wqk")
            wvs = big.tile([128, CC, HPC * HD], F16, tag="wv")
            xTs = big.tile([128, CC, T], F16, tag="xT")
            bqks = big.tile([128, 3], F32, tag="bqk")
            wos01 = big.tile([128, D], F16, tag="wo01")
            wos2 = big.tile([HD, D], F16, tag="wo2")
            QQ = big.tile([128, T], F16, tag="QQ")
            KK = big.tile([128, T], F16, tag="KK")
            QQ2 = big.tile([HD, T], F16, tag="QQ2")
            KK2 = big.tile([HD, T], F16, tag="KK2")
            # token-major V (+ ones column at 64): [128, kc, h, 66]
            vT3 = big.tile([128, KC, HPC, 66], F16, tag="vT3")
            ynA = big.tile([128, T], F16, tag="ynA")
            ynB = big.tile([HD, T], F16, tag="ynB")
            mask_tri = big.tile([128, 128], F16, tag="mask")
            ones_t = big.tile([128, HD], F32R, tag="ones")

            # PE p-state warm-up: a dense run of ~free N=1 matmuls starts
            # the tensor engine's ramp clock during the input-DMA wait so
            # the real matmuls reach full clock ~2us earlier; the dummy Exp
            # pulls the activation-table load (1.3us) off the first real
            # exp's critical path.
            wtiny = big.tile([1, 8], F16, tag="wtiny")
            nc.vector.memset(wtiny, 0.5)
            nc.scalar.activation(out=wtiny[0:1, 4:8], in_=wtiny[0:1, 0:4],
                                 func=mybir.ActivationFunctionType.Exp)
            s_warm = psS.tile([128, EXP_BATCH, 512], F32, tag="s3",
                              name="s_warm")
            warm_cols = 1024
            for i in range(550):
                j = i % warm_cols
                nc.tensor.matmul(
                    s_warm[0:1, j // 512, j % 512:j % 512 + 1],
                    lhsT=wtiny[0:1, 0:1], rhs=wtiny[0:1, 0:1],
                    start=True, stop=True, skip_group_check=True)

            # ---- input DMAs: window-0 criticals first. wqk goes through
            # the SP HWDGE queue while xT window-0 chunks go through the
            # Pool SWDGE queue -- two parallel descriptor-generation paths.
            nc.gpsimd.dma_start(out=wqks[:, 0, 0:1], in_=wqk[0, :, 0:1])
            nc.gpsimd.dma_start(out=xTs[:, 3, 0:512], in_=xT[3, :, 0:512])
            nc.gpsimd.dma_start(out=xTs[:, 4, 0:512], in_=xT[4, :, 0:512])
            nc.gpsimd.dma_start(out=xTs[:, 5, 0:512], in_=xT[5, :, 0:512])
            nc.gpsimd.dma_start(out=wqks[:, 5], in_=wqk[5])
            # the GPSIMD ucode library carrying partition_broadcast loads
            # after the startup DMAs so it doesn't gate them
            nc.gpsimd.load_library(library_config.proxy)
            nc.sync.dma_start(out=xTs[:, 0, 0:512], in_=xT[0, :, 0:512])
            nc.sync.dma_start(out=xTs[:, 1, 0:512], in_=xT[1, :, 0:512])
            nc.sync.dma_start(out=wqks[:, 0, 1:3], in_=wqk[0, :, 1:3])
            nc.sync.dma_start(out=wqks[:, 1], in_=wqk[1])
            nc.sync.dma_start(out=xTs[:, 2, 0:512], in_=xT[2, :, 0:512])
            nc.sync.dma_start(out=wqks[:, 2], in_=wqk[2])
            nc.sync.dma_start(out=wvs[:, 0:3], in_=wv[0:3].rearrange("c p f -> p c f"))
            nc.sync.dma_start(out=wvs[:, 3:6], in_=wv[3:6].rearrange("c p f -> p c f"))
            nc.sync.dma_start(out=wqks[:, 3], in_=wqk[3])
            nc.sync.dma_start(out=wqks[:, 4], in_=wqk[4])
            nc.sync.dma_start(out=bqks, in_=bqk[:, :])
            for cc in range(CC):
                nc.sync.dma_start(out=xTs[:, cc, 512:T],
                                  in_=xT[cc, :, 512:T])
                if cc == 2:
                    nc.sync.dma_start(out=wos01, in_=wo01d[:, :])
                    nc.sync.dma_start(out=wos2, in_=wo2d[:, :])

            # ---- constants ----
            make_upper_triangular(nc, mask_tri, val=1.0, diag=True)
            ones_stage = big.tile([128, HD], F32, tag="ones_stage")
            nc.vector.memset(ones_stage, 1.0)
            with nc.allow_low_precision(reason="fp32r ones for normalizer "
                                        "broadcast matmul"):
                nc.vector.tensor_copy(out=ones_t, in_=ones_stage)
            for h in range(HPC):
                nc.gpsimd.memset(vT3[:, :, h, HD:HD + 1], 1.0)

            st = {
                "pending_norm": [],
                "wqks": wqks, "wvs": wvs, "xTs": xTs, "bqks": bqks,
                "wos01": wos01, "wos2": wos2, "QQ": QQ, "KK": KK,
                "QQ2": QQ2, "KK2": KK2, "vT3": vT3, "ynA": ynA, "ynB": ynB,
                "mask": mask_tri, "ones": ones_t,
                "psS": psS, "psY": psY, "psA": psA,
                "ets": ets, "scr": scr, "outs": outs, "outT": outT,
            }

            # prologue: window-0 q/k projection, cc-major across three
            # psY accumulators so each arriving x chunk is consumed
            # immediately; V chunks 0/1 here, 2/3 ride window 0's bg queue.
            paccs = [psY.tile([128, 512], F32, tag="y", name=f"pacc_f{fc}")
                     for fc in range(3)]
            for cc in range(CC):
                for fc in range(3):
                    nc.tensor.matmul(
                        paccs[fc],
                        lhsT=wqks[:, cc, fc, :],
                        rhs=xTs[:, cc, 0:512],
                        start=(cc == 0), stop=(cc == CC - 1),
                    )
            for fc in range(3):
                _evac_qk(nc, st, 0, fc, paccs[fc])
            _proj_v_chunk(nc, st, 0)
            _proj_v_chunk(nc, st, 1, acc="s3")

            for w in range(TW):
                _attn_window(nc, st, w)

            # epilogue: output projection for the last window, split-phase:
            # the h0/h1 contraction runs over 6 parallel accumulators
            # (borrowing the now-idle psS/psY banks) while the h2 stream's
            # normalize drains, then the h2 matmuls close each group.
            qs = (TW - 1) * 512
            opst = []
            for ec in range(CC):
                pool, tag = [(psY, "y"), (psY, "y"), (psY, "y"),
                             (psS, "s3"), (psS, "s3"), (psA, "acc")][ec]
                opst.append(pool.tile([128, 512], F32, tag=tag,
                                      name=f"opse_{ec}"))
            osb6e = outs.tile([128, CC, 512], F16, tag="osb",
                              name="osb_epi")
            for ec in range(CC):
                for c0 in (0, 256):
                    nc.tensor.matmul(
                        opst[ec][:, c0:c0 + 256],
                        lhsT=wos01[:, ec * 128:(ec + 1) * 128],
                        rhs=ynA[:, qs + c0:qs + c0 + 256],
                        start=(c0 == 0), stop=False, skip_group_check=True,
                    )
                for c0 in (0, 256):
                    nc.tensor.matmul(
                        opst[ec][:, c0:c0 + 256],
                        lhsT=wos2[:, ec * 128:(ec + 1) * 128],
                        rhs=ynB[:, qs + c0:qs + c0 + 256],
                        start=False, stop=True, skip_group_check=True,
                    )
                if ec % 2 == 1:
                    nc.vector.tensor_copy(out=osb6e[:, ec, :], in_=opst[ec])
                else:
                    nc.scalar.copy(out=osb6e[:, ec, :], in_=opst[ec])
                if ec == 3:
                    nc.sync.dma_start(
                        out=outT[0:512, qs:qs + 512].rearrange(
                            "(e p) c -> p e c", e=4),
                        in_=osb6e[:, 0:4, :],
                    )
                elif ec == 5:
                    nc.scalar.dma_start(
                        out=outT[512:768, qs:qs + 512].rearrange(
                            "(e p) c -> p e c", e=2),
                        in_=osb6e[:, 4:6, :],
                    )
    return nc


def _proj_qk_window(nc, st, w):
    """q/k projection for token window w: 3 fc groups of 6 matmuls each,
    bias folded into the DVE evacuation."""
    ts_ = w * 512
    for fc in range(3):
        _proj_qk_group(nc, st, w, fc)
    del ts_


def _proj_qk_group(nc, st, w, fc, acc="acc"):
    ts_ = w * 512
    pool = st["psY"] if acc == "y" else st["psA"]
    ps = pool.tile([128, 512], F32, tag=acc, name=f"ps_f{fc}_t{w}")
    for cc in range(CC):
        nc.tensor.matmul(
            ps,
            lhsT=st["wqks"][:, cc, fc, :],
            rhs=st["xTs"][:, cc, ts_:ts_ + 512],
            start=(cc == 0), stop=(cc == CC - 1),
        )
    _evac_qk(nc, st, w, fc, ps)


def _evac_qk(nc, st, w, fc, ps):
    ts_ = w * 512
    # evacuate with bias add (per-partition scalar).
    # fc0 = [h0q|h1q] -> QQ; fc1 = [h2q|h2k] -> QQ2/KK2; fc2 = [h0k|h1k] -> KK
    if fc == 1 and w == 0:
        # prologue: the four evacuations gate window 0's first scores and
        # serialize on DVE; run fc1's pair on the still-idle Act engine
        nc.scalar.activation(
            out=st["QQ2"][:, ts_:ts_ + 512], in_=ps[0:HD, :],
            func=mybir.ActivationFunctionType.Identity,
            bias=st["bqks"][0:HD, 1:2])
        nc.scalar.activation(
            out=st["KK2"][:, ts_:ts_ + 512], in_=ps[HD:128, :],
            func=mybir.ActivationFunctionType.Identity,
            bias=st["bqks"][HD:128, 1:2])
        return
    if fc == 0:
        nc.vector.tensor_scalar(
            out=st["QQ"][:, ts_:ts_ + 512], in0=ps,
            scalar1=st["bqks"][:, fc:fc + 1], scalar2=None,
            op0=mybir.AluOpType.add)
    elif fc == 2:
        nc.vector.tensor_scalar(
            out=st["KK"][:, ts_:ts_ + 512], in0=ps,
            scalar1=st["bqks"][:, fc:fc + 1], scalar2=None,
            op0=mybir.AluOpType.add)
    else:
        nc.vector.tensor_scalar(
            out=st["QQ2"][:, ts_:ts_ + 512], in0=ps[0:HD, :],
            scalar1=st["bqks"][0:HD, fc:fc + 1], scalar2=None,
            op0=mybir.AluOpType.add)
        nc.vector.tensor_scalar(
            out=st["KK2"][:, ts_:ts_ + 512], in0=ps[HD:128, :],
            scalar1=st["bqks"][HD:128, fc:fc + 1], scalar2=None,
            op0=mybir.AluOpType.add)


def _proj_v_chunk(nc, st, tc_i, acc="acc"):
    """token-major V projection for one 128-token chunk."""
    pool = {"y": st["psY"], "s3": st["psS"],
            "acc": st["psA"]}[acc]
    psv = pool.tile([128, 512], F32, tag=acc, name=f"psv_{tc_i}")
    for cc in range(CC):
        nc.tensor.matmul(
            psv[:, 0:HPC * HD],
            lhsT=st["xTs"][:, cc, tc_i * 128:(tc_i + 1) * 128],
            rhs=st["wvs"][:, cc, :],
            start=(cc == 0), stop=(cc == CC - 1),
        )
    nc.vector.tensor_copy(
        out=st["vT3"][:, tc_i, :, 0:HD],
        in_=psv[:, 0:HPC * HD].rearrange("p (h d) -> p h d", h=HPC),
    )


def _outproj_group(nc, st, w, ec):
    qs = w * 512
    act_ok = w < TW - 2  # evacs run in window w+1; Act has slack if w+1<=2
    if ec == 0:
        st["osb6"] = st["outs"].tile([128, CC, 512], F16, tag="osb",
                                     name=f"osb_q{w}")
    ops = st["psA"].tile([128, 512], F32, tag="acc", name=f"ops_e{ec}_q{w}")
    nc.tensor.matmul(
        ops,
        lhsT=st["wos01"][:, ec * 128:(ec + 1) * 128],
        rhs=st["ynA"][:, qs:qs + 512],
        start=True, stop=False,
    )
    nc.tensor.matmul(
        ops,
        lhsT=st["wos2"][:, ec * 128:(ec + 1) * 128],
        rhs=st["ynB"][:, qs:qs + 512],
        start=False, stop=True,
    )
    if w < TW - 2 and ec % 2 == 1:
        nc.scalar.copy(out=st["osb6"][:, ec, :], in_=ops)
    else:
        nc.vector.tensor_copy(out=st["osb6"][:, ec, :], in_=ops)
    if ec == CC - 1:
        nc.sync.dma_start(
            out=st["outT"][:, qs:qs + 512].rearrange(
                "(e p) c -> p e c", e=CC),
            in_=st["osb6"],
        )


def _attn_window(nc, st, w):
    """Attention for q-window w across the 3 head streams, with background
    PE work (next-window projection, previous-window outproj) interleaved
    between score/PV rounds."""
    qs = w * 512
    nchunks = 4 * (w + 1)
    # below-diagonal chunks first (descending, so the first PV write is the
    # full column range), diagonal chunks last: the window's own K-side
    # projection (fc2) and V chunks then slide into this window's early
    # rounds instead of crowding the previous one.
    kc_order = list(range(4 * w))[::-1] + list(range(4 * w, nchunks))
    batches = [kc_order[i:i + EXP_BATCH]
               for i in range(0, nchunks, EXP_BATCH)]
    n_diag_batches = 2

    # bg_must: work that must land before this window's diagonal rounds.
    # bg_opt: deferrable work (previous window's normalize phase B and
    # outproj, next window's Q-side projection).
    bg_must = []
    if w == 0:
        for j in (2, 3):
            bg_must.append(lambda j=j: _proj_v_chunk(nc, st, j))
    else:
        bg_must.append(lambda: _proj_qk_group(nc, st, w, 2))
        for j in range(4):
            bg_must.append(lambda j=j: _proj_v_chunk(nc, st, 4 * w + j))
    bg_opt = []
    for s_, w_ in st.pop("pending_norm", []):
        bg_opt.append(lambda s_=s_, w_=w_: _normalize_b(nc, st, s_, w_))
    if w + 1 < TW:
        for fc in (0, 1):
            bg_opt.append(lambda fc=fc: _proj_qk_group(nc, st, w + 1, fc))
    if w >= 1:
        for ec in range(CC):
            bg_opt.append(lambda ec=ec: _outproj_group(nc, st, w - 1, ec))

    n_below = len(batches) - n_diag_batches

    def bg_slot(diag_phase=False):
        if bg_must:
            bg_must.pop(0)()
        elif bg_opt:
            # hold back two opt groups to feed the PE during the
            # Act-bound diagonal rounds
            if diag_phase or len(bg_opt) > 2 or w == 0:
                bg_opt.pop(0)()

    bg = bg_opt  # leftover drain at window end uses the opt queue

    streams = [
        {"h": 0, "qq": st["QQ"], "kk": st["KK"], "rb": 0,
         "yn_ap": lambda q, n: st["ynA"][0:HD, q:q + n]},
        {"h": 1, "qq": st["QQ"], "kk": st["KK"], "rb": HD,
         "yn_ap": lambda q, n: st["ynA"][HD:128, q:q + n]},
        {"h": 2, "qq": st["QQ2"], "kk": st["KK2"], "rb": 0,
         "yn_ap": lambda q, n: st["ynB"][0:HD, q:q + n]},
    ]
    for s in streams:
        s["y"] = st["psY"].tile([128, 512], F32, tag="y",
                                name=f"y_h{s['h']}_q{w}")

    for bi, kcs in enumerate(batches):
        nb = len(kcs)
        diag_phase = bi >= len(batches) - n_diag_batches
        if bi == len(batches) - n_diag_batches:
            while bg_must:
                bg_must.pop(0)()
        # --- scores + exp for all 3 streams; bg slice between h1 and h2 ---
        ebt = {}
        for si, s in enumerate(streams):
            if si == 2:
                bg_slot(diag_phase)
            h = s["h"]
            rb = s["rb"]
            schrau = (SCHRAU and w == TW - 1 and h == 2
                      and all(kc < 4 * w for kc in kcs))
            s_ps = st["psS"].tile([128, EXP_BATCH, 512], F32, tag="s3",
                                  name=f"s_h{h}_q{w}_b{bi}")
            if schrau:
                eti = st["ets"].tile([128, EXP_BATCH, 512], mybir.dt.int16,
                                     tag="et", name=f"e_h{h}_q{w}_b{bi}")
                et = eti.bitcast(mybir.dt.bfloat16)
            else:
                et = st["ets"].tile([128, EXP_BATCH, 512], F16, tag="et",
                                    name=f"e_h{h}_q{w}_b{bi}")
            ebt[h] = et
            js = [max(0, kc - 4 * w) for kc in kcs]
            jw = js if w in (1, 2) else [min(js)] * nb
            for i in range(nb):
                kc = kcs[i]
                j = jw[i]
                nc.tensor.matmul(
                    s_ps[:, i, 128 * j:512],
                    lhsT=s["kk"][rb:rb + HD, kc * 128:(kc + 1) * 128],
                    rhs=s["qq"][rb:rb + HD, qs + 128 * j:qs + 512],
                    start=True, stop=True,
                )
            if schrau:
                nc.vector.tensor_scalar(
                    out=eti[:, 0:nb, :], in0=s_ps[:, 0:nb, :],
                    scalar1=SCHRAU_A * SCALE, scalar2=SCHRAU_B,
                    op0=mybir.AluOpType.mult, op1=mybir.AluOpType.add)
                continue
            if any(js) and w in (1, 2):
                # ragged diagonal batch: exp per chunk over exactly the
                # region its score matmul wrote
                for i in range(nb):
                    nc.scalar.activation(
                        out=et[:, i, 128 * js[i]:512],
                        in_=s_ps[:, i, 128 * js[i]:512],
                        func=mybir.ActivationFunctionType.Exp, scale=SCALE,
                    )
            else:
                jm = min(js)
                nc.scalar.activation(
                    out=et[:, 0:nb, 128 * jm:512],
                    in_=s_ps[:, 0:nb, 128 * jm:512],
                    func=mybir.ActivationFunctionType.Exp, scale=SCALE,
                )
            for i in range(nb):
                j = kcs[i] - 4 * w
                if j < 0:
                    continue
                nc.gpsimd.tensor_mul(
                    out=et[:, i, 128 * j:128 * (j + 1)],
                    in0=et[:, i, 128 * j:128 * (j + 1)],
                    in1=st["mask"],
                )
        # --- PV for all 3 streams; bg slice between h1 and h2; on the
        # last round each stream's normalize follows its last PV so the
        # normalize chains overlap the remaining streams' PE work ---
        last_round = bi == len(batches) - 1
        next_last = bi == len(batches) - 2
        for si, s in enumerate(streams):
            if si == 2:
                bg_slot(diag_phase)
            et = ebt[s["h"]]
            for i in range(nb):
                kc = kcs[i]
                j = max(0, kc - 4 * w)
                idx = bi * EXP_BATCH + i
                nc.tensor.matmul(
                    s["y"][0:HD + 1, 128 * j:512],
                    lhsT=st["vT3"][:, kc, s["h"], 0:HD + 1],
                    rhs=et[:, i, 128 * j:512],
                    start=(idx == 0), stop=(idx == nchunks - 1),
                    skip_group_check=True,
                )
            if w == TW - 1:
                # the diagonal chunks only touch ascending column ranges, so
                # sumexp[0:256] is final one batch early: run the normalize
                # chain per column half as it becomes final, overlapping the
                # last batch's score/exp/PV work
                if next_last:
                    _normalize_half(nc, st, s, w, 0)
                elif last_round:
                    _normalize_half(nc, st, s, w, 256)
            elif last_round:
                _normalize_a(nc, st, s, w)
    if w != TW - 1:
        st["pending_norm"] = [(s, w) for s in streams]

    # leftover background groups
    while bg:
        bg.pop(0)()


def _normalize_half(nc, st, s, w, c0):
    """full normalize chain for one 256-wide column half (last window)."""
    qs = w * 512
    h = s["h"]
    y = s["y"]
    if c0 == 0:
        s["sc"] = st["scr"].tile([128, 512], F32, tag="sc",
                                 name=f"sc_h{h}_q{w}")
        s["rbt"] = st["scr"].tile([HD, 512], F32, tag="rbs",
                                  name=f"rb_h{h}_q{w}")
    sc, rb = s["sc"], s["rbt"]
    nc.vector.reciprocal(out=sc[0:1, c0:c0 + 256],
                         in_=y[HD:HD + 1, c0:c0 + 256])
    nc.gpsimd.partition_broadcast(rb[:, c0:c0 + 256], sc[0:1, c0:c0 + 256])
    nc.vector.tensor_mul(
        out=s["yn_ap"](qs + c0, 256),
        in0=y[0:HD, c0:c0 + 256], in1=rb[:, c0:c0 + 256],
    )


def _normalize_a(nc, st, s, w):
    """reciprocal of the sumexp row (column halves on the last window so
    phase B can start earlier; full width otherwise)."""
    h = s["h"]
    y = s["y"]
    sc = st["scr"].tile([128, 512], F32, tag="sc", name=f"sc_h{h}_q{w}")
    s["sc"] = sc
    halves = (0, 256) if w == TW - 1 else (0,)
    wd = 256 if w == TW - 1 else 512
    with nc.allow_low_precision(reason="fp32r == fp32 bits; rounding "
                                "only affects the PE broadcast matmul"):
        for c0 in halves:
            nc.vector.reciprocal(out=sc[0:1, c0:c0 + wd],
                                 in_=y[HD:HD + 1, c0:c0 + wd])


def _normalize_b(nc, st, s, w):
    """broadcast 1/sumexp into the y tile's free partitions 64..127 via a
    K=1 fp32r matmul, then y[0:64] * y[64:128] -> yn (two column halves so
    the output projection can start on the first half early)."""
    qs = w * 512
    h = s["h"]
    y = s["y"]
    sc = s["sc"]
    rb = st["scr"].tile([HD, 512], F32, tag="rbs", name=f"rb_h{h}_q{w}")
    halves = (0, 256) if w == TW - 1 else (0,)
    wd = 256 if w == TW - 1 else 512
    # broadcast 1/sumexp from sc partition 0 to 64 partitions on the GPSIMD
    # engine (SBUF->SBUF; PSUM matmul outputs can't start at partition 64,
    # and the psA bank is contended by background groups)
    for c0 in halves:
        nc.gpsimd.partition_broadcast(rb[:, c0:c0 + wd],
                                      sc[0:1, c0:c0 + wd])
    for c0 in halves:
        nc.vector.tensor_mul(
            out=s["yn_ap"](qs + c0, wd),
            in0=y[0:HD, c0:c0 + wd], in1=rb[:, c0:c0 + wd],
        )


def _prep_core_inputs(c, x, w_qkv, b_qkv, w_out):
    b = c // CPB
    g = c % CPB
    hs = [HPC * g + i for i in range(HPC)]

    qc = [np.arange(h * HD, (h + 1) * HD) for h in hs]
    kc_ = [D + h * HD + np.arange(HD) for h in hs]
    vc = [2 * D + h * HD + np.arange(HD) for h in hs]

    cols = np.concatenate([qc[0], qc[1], qc[2], kc_[2], kc_[0], kc_[1]])
    vcols = np.concatenate(vc)

    xT = np.ascontiguousarray(x[b].T)
    return {
        "xT": np.ascontiguousarray(
            xT.astype(np.float16).reshape(CC, 128, T)),
        "wqk": np.ascontiguousarray(
            w_qkv[:, cols].astype(np.float16).reshape(CC, 128, 3, 128)),
        "bqk": np.ascontiguousarray(
            b_qkv[cols].reshape(3, 128).T.astype(np.float32)),
        "wv": np.ascontiguousarray(
            w_qkv[:, vcols].astype(np.float16).reshape(CC, 128, HPC * HD)),
        "wo01": np.ascontiguousarray(
            w_out[192 * g:192 * g + 128, :].astype(np.float16)),
        "wo2": np.ascontiguousarray(
            w_out[192 * g + 128:192 * g + 192, :].astype(np.float16)),
    }


_NC_CACHE = {}


def get_nc():
    if "nc" not in _NC_CACHE:
        nc = build_bass()
        nc.finalize()
        _NC_CACHE["nc"] = nc
    return _NC_CACHE["nc"]


def kernel(x, w_qkv, b_qkv, w_out, b_out, _run_kwargs=None):
    x = np.asarray(x, dtype=np.float32)
    w_qkv = np.asarray(w_qkv, dtype=np.float32)
    b_qkv = np.asarray(b_qkv, dtype=np.float32)
    w_out = np.asarray(w_out, dtype=np.float32)
    b_out = np.asarray(b_out, dtype=np.float32)

    nc = get_nc()
    in_maps = [_prep_core_inputs(c, x, w_qkv, b_qkv, w_out)
               for c in range(NCORES)]
    kwargs = dict(_run_kwargs or {})
    res = run_bass_kernel_spmd(nc, in_maps, core_ids=list(range(NCORES)),
                               **kwargs)
    if kwargs:
        _NC_CACHE["last_results"] = res

    bv_corr = b_qkv[2 * D:3 * D] @ w_out  # [D]
    out = np.zeros((B, T, D), dtype=np.float32)
    for b in range(B):
        acc = np.zeros((T, D), dtype=np.float32)
        for g in range(CPB):
            acc += np.asarray(res.results[b * CPB + g]["outT"]
                              ).astype(np.float32).T
        out[b] = acc + (b_out + bv_corr)[None, :]
    return out


if __name__ == "__main__":
    nc = build_bass()
    print("built OK")



# revision 61
# speedup vs baseline: 1.1704x; 1.1704x over previous
"""Causal self-attention (B=2, T=2048, D=768, H=12) on 8 TRN2 NeuronCores.

Sharding: tensor-parallel over (batch, head) pairs; 3 heads per core, one
batch per 4-core group; the host sums the 4 partial outputs per batch and
adds b_out (+ the v-bias correction through w_out).

fp8 strategy (the PE cost model charges output-columns x cycles/row, with
fp8e4+DoubleRow at 0.5 cycles/row and contraction depth free):
- QKV / V projections run as 3-term residual fp8 DoubleRow matmuls:
  x = xh + xl (hi/lo fp8), w*64 = wh + wl, and
  psum = xh@wh + xh@wl + xl@wh (all same scale, lo*lo dropped, ~0.1% err).
  DoubleRow pairs adjacent contraction chunks ([128,2,*] APs), so the
  6-chunk contraction is 9 DR matmuls at half rate (vs 6 full-rate fp16).
- Scores run as fp8 DoubleRow with the head dim split 2x32: Q/K are
  evacuated to a flat fp8 tile (Q gets +bias*64; K's bias is dropped -- it
  is constant along the softmax axis, hence exactly softmax-invariant),
  then one SBUF->SBUF DMA per fc-group splits [128,512] into the
  [32,2,slot,512] layout (d = 2p+i interleave, heads pre-interleaved in
  the weight column order by the host).
- exp scale absorbs the 64*64 weight scaling: exp(s * SCALE/4096).
- PV and the output projection stay fp16 (attention-weight and value
  precision bound the error budget; measured ~1e-2 end-to-end).

Per 512-token q-window, the scores->exp->PV chain runs in "rounds"
of 2-k-chunk batches across the 3 head streams; below-diagonal chunks run
first, and background PE work (the window's K-side projection + V chunks,
the next window's Q-side projection, the previous window's output
projection, deferred normalize phases) is interleaved between rounds so
the PE keeps running while ScalarE exp latency drains. The softmax
normalizer uses a reciprocal on DVE plus a GPSIMD partition_broadcast.
A warm-up run of free N=1 matmuls starts the PE p-state ramp during the
initial DMA wait.

Constraints learned on real silicon (CoreSim does not model them):
GPSIMD cannot access PSUM at all; tensor_tensor cannot read two PSUM
operands; PSUM matmul outputs must start at partition 0.
"""

import numpy as np
import ml_dtypes

import concourse.bass as bass
import concourse.bacc as bacc
import concourse.mybir as mybir
import concourse.tile as tile
from concourse import library_config
from concourse.masks import make_upper_triangular
from concourse.bass_utils import run_bass_kernel_spmd

B, T, D, H, HD = 2, 2048, 768, 12, 64
NCORES = 8
HPC = 3            # heads per core
CPB = NCORES // B  # cores per batch = 4
CC = D // 128      # d_model chunks of 128 = 6
TW = T // 512      # token windows of 512 = 4
KC = T // 128      # k chunks of 128 = 16
SCALE = 1.0 / float(np.sqrt(HD))
WSC = 64.0         # fp8 weight pre-scale (folded into the exp scale / evacs)
ESCALE = SCALE / (WSC * WSC)   # exp scale for raw fp8-score psums

F8 = mybir.dt.float8e4
F16 = mybir.dt.float16
F32 = mybir.dt.float32
E4NP = ml_dtypes.float8_e4m3fn
DR = mybir.MatmulPerfMode.DoubleRow

EXP_BATCH = 2  # k-chunks per exp call / per s-tile (PSUM tile = 2 banks)

# QKD layout [32, fc, i, slot, T] (i = d-pair index, slot = head lane):
# per head h: q at (fcg, sl) and k at (fcg, sl) below
QPOS = ((0, 0), (0, 1), (1, 0))   # h0, h1, h2
KPOS = ((2, 0), (2, 1), (1, 1))

# Schraudolph fast-exp on DVE for stream h2's below-diagonal batches in
# the last window (relieves the ScalarE bottleneck there). exp(x) ~ bf16
# bitcast of int16(A*x + B); ~2.4% RMS approximation error on ~11% of the
# attention weights.
SCHRAU = True
SCHRAU_A = 128.0 / np.log(2.0)
SCHRAU_B = float(127 << 7) - 7.5


def build_bass():
    nc = bacc.Bacc(None, target_bir_lowering=False)

    xh = nc.dram_tensor("xh", [CC, 128, T], F8, kind="ExternalInput")
    xl = nc.dram_tensor("xl", [CC, 128, T], F8, kind="ExternalInput")
    wqh = nc.dram_tensor("wqh", [CC, 128, 3, 128], F8, kind="ExternalInput")
    wql = nc.dram_tensor("wql", [CC, 128, 3, 128], F8, kind="ExternalInput")
    bqk = nc.dram_tensor("bqk", [128, 3], F32, kind="ExternalInput")
    wvh = nc.dram_tensor("wvh", [CC, 128, HPC * HD], F8, kind="ExternalInput")
    wvl = nc.dram_tensor("wvl", [CC, 128, HPC * HD], F8, kind="ExternalInput")
    wo01d = nc.dram_tensor("wo01", [128, D], F16, kind="ExternalInput")
    wo2d = nc.dram_tensor("wo2", [HD, D], F16, kind="ExternalInput")
    outT = nc.dram_tensor("outT", [D, T], F16, kind="ExternalOutput")

    with tile.TileContext(nc) as tc:
        with (
            tc.tile_pool(name="big", bufs=1) as big,
            tc.tile_pool(name="ets", bufs=6) as ets,
            tc.tile_pool(name="scr", bufs=3) as scr,
            tc.tile_pool(name="outs", bufs=4) as outs,
            tc.tile_pool(name="psS", bufs=2, space="PSUM") as psS,
            tc.tile_pool(name="psY", bufs=3, space="PSUM") as psY,
            tc.tile_pool(name="psA", bufs=1, space="PSUM") as psA,
        ):
            # ---- SBUF persistent tiles ----
            wqhs = big.tile([128, CC, 3, 128], F8, tag="wqh")
            wqls = big.tile([128, CC, 3, 128], F8, tag="wql")
            wvhs = big.tile([128, CC, HPC * HD], F8, tag="wvh")
            wvls = big.tile([128, CC, HPC * HD], F8, tag="wvl")
            xhs = big.tile([128, CC, T], F8, tag="xh")
            xls = big.tile([128, CC, T], F8, tag="xl")
            bqks = big.tile([128, 3], F32, tag="bqk")
            wos01 = big.tile([128, D], F16, tag="wo01")
            wos2 = big.tile([HD, D], F16, tag="wo2")
            QKF = big.tile([128, 3, T], F8, tag="QKF")      # flat staging
            QKD = big.tile([32, 3, 2, 2, T], F8, tag="QKD")  # DR split layout
            # token-major V (+ ones column at 64): [128, kc, h, 66]
            vT3 = big.tile([128, KC, HPC, 66], F16, tag="vT3")
            ynA = big.tile([128, T], F16, tag="ynA")
            ynB = big.tile([HD, T], F16, tag="ynB")
            mask_tri = big.tile([128, 128], F16, tag="mask")

            # PE p-state warm-up: a dense run of ~free N=1 matmuls starts
            # the tensor engine's ramp clock during the input-DMA wait so
            # the real matmuls reach full clock ~2us earlier; the dummy Exp
            # pulls the activation-table load (1.3us) off the first real
            # exp's critical path.
            wtiny = big.tile([1, 8], F16, tag="wtiny")
            nc.vector.memset(wtiny, 0.5)
            nc.scalar.activation(out=wtiny[0:1, 4:8], in_=wtiny[0:1, 0:4],
                                 func=mybir.ActivationFunctionType.Exp)
            s_warm = psS.tile([128, EXP_BATCH, 512], F32, tag="s3",
                              name="s_warm")
            warm_cols = 1024
            for i in range(120):
                j = i % warm_cols
                nc.tensor.matmul(
                    s_warm[0:1, j // 512, j % 512:j % 512 + 1],
                    lhsT=wtiny[0:1, 0:1], rhs=wtiny[0:1, 0:1],
                    start=True, stop=True, skip_group_check=True)

            # ---- input DMAs: window-0 criticals first. Weights go through
            # the Pool SWDGE queue while x window-0 chunks go through the
            # SP HWDGE queue -- two parallel descriptor-generation paths.
            nc.gpsimd.dma_start(
                out=wqhs[:, 0:2], in_=wqh[0:2].rearrange("c p f k -> p c f k"))
            nc.gpsimd.dma_start(out=xhs[:, 4, 0:512], in_=xh[4, :, 0:512])
            nc.gpsimd.dma_start(out=xhs[:, 5, 0:512], in_=xh[5, :, 0:512])
            nc.gpsimd.dma_start(
                out=wqhs[:, 4:6], in_=wqh[4:6].rearrange("c p f k -> p c f k"))
            # the GPSIMD ucode library carrying partition_broadcast loads
            # after the startup DMAs so it doesn't gate them
            nc.gpsimd.load_library(library_config.proxy)
            # SP queue: x hi window-0 + the q/k weight pair not on Pool
            nc.sync.dma_start(
                out=xhs[:, 0:2, 0:512],
                in_=xh[0:2, :, 0:512].rearrange("c p t -> p c t"))
            nc.sync.dma_start(
                out=wqhs[:, 2:4], in_=wqh[2:4].rearrange("c p f k -> p c f k"))
            nc.sync.dma_start(
                out=xhs[:, 2:4, 0:512],
                in_=xh[2:4, :, 0:512].rearrange("c p t -> p c t"))
            nc.sync.dma_start(out=bqks, in_=bqk[:, :])
            # Act queue (idle at startup): lo-residual inputs; V weights ride
            # the Pool SWDGE queue
            nc.scalar.dma_start(
                out=wqls, in_=wql.rearrange("c p f k -> p c f k"))
            for c0 in (0, 2, 4):
                nc.scalar.dma_start(
                    out=xls[:, c0:c0 + 2, 0:512],
                    in_=xl[c0:c0 + 2, :, 0:512].rearrange("c p t -> p c t"))
            nc.gpsimd.dma_start(
                out=wvhs, in_=wvh.rearrange("c p f -> p c f"))
            nc.gpsimd.dma_start(
                out=wvls, in_=wvl.rearrange("c p f -> p c f"))
            # x tails: window-1 token range first so window-1 projections
            # (window-0 bg slots) aren't input-starved; single big DMAs to
            # keep the SP issue queue short
            nc.sync.dma_start(
                out=xhs[:, :, 512:1024],
                in_=xh[:, :, 512:1024].rearrange("c p t -> p c t"))
            nc.sync.dma_start(
                out=xls[:, :, 512:1024],
                in_=xl[:, :, 512:1024].rearrange("c p t -> p c t"))


            # ---- constants ----
            make_upper_triangular(nc, mask_tri, val=1.0, diag=True)
            for h in range(HPC):
                nc.gpsimd.memset(vT3[:, :, h, HD:HD + 1], 1.0)

            st = {
                "pending_norm": [],
                "wqhs": wqhs, "wqls": wqls, "wvhs": wvhs, "wvls": wvls,
                "xhs": xhs, "xls": xls, "bqks": bqks,
                "wos01": wos01, "wos2": wos2, "QKF": QKF, "QKD": QKD,
                "vT3": vT3, "ynA": ynA, "ynB": ynB,
                "mask": mask_tri,
                "psS": psS, "psY": psY, "psA": psA,
                "ets": ets, "scr": scr, "outs": outs, "outT": outT,
            }

            # prologue: window-0 q/k projection. 3 psY accumulators, one per
            # fc group; term-major (hh, hl, lh) so each arriving input is
            # consumed immediately; V chunks 0/1 here, 2/3 ride window 0's
            # bg queue.
            paccs = [psY.tile([128, 512], F32, tag="y", name=f"pacc_f{fc}")
                     for fc in range(3)]
            terms = [(wqhs, xhs), (wqls, xhs), (wqhs, xls)]
            for ti, (wt, xt) in enumerate(terms):
                for c0 in (0, 2, 4):
                    for fc in range(3):
                        nc.tensor.matmul(
                            paccs[fc],
                            lhsT=wt[:, c0:c0 + 2, fc, :],
                            rhs=xt[:, c0:c0 + 2, 0:512],
                            start=(ti == 0 and c0 == 0),
                            stop=(ti == 2 and c0 == 4),
                            perf_mode=DR,
                        )
            # spread window-0's three split DMAs across three queues so
            # their descriptor generations run in parallel
            _evac_qk(nc, st, 0, 0, paccs[0], act=False, split=False)
            _evac_qk(nc, st, 0, 1, paccs[1], act=True, split=False)
            _evac_qk(nc, st, 0, 2, paccs[2], act=True, split=False)
            _split_qk(nc, st, 0, 0, queue=nc.sync)
            _split_qk(nc, st, 0, 2, queue=nc.scalar)
            _split_qk(nc, st, 0, 1, queue=nc.gpsimd)
            _proj_v_chunk(nc, st, 0)
            _proj_v_chunk(nc, st, 1, acc="s3")
            nc.sync.dma_start(
                out=xhs[:, :, 1024:T],
                in_=xh[:, :, 1024:T].rearrange("c p t -> p c t"))
            nc.sync.dma_start(out=wos01, in_=wo01d[:, :])
            nc.sync.dma_start(out=wos2, in_=wo2d[:, :])
            nc.sync.dma_start(
                out=xls[:, :, 1024:T],
                in_=xl[:, :, 1024:T].rearrange("c p t -> p c t"))

            for w in range(TW):
                _attn_window(nc, st, w)

            # epilogue: output projection for the last window
            qs = (TW - 1) * 512
            opst = []
            for ec in range(CC):
                pool, tag = [(psY, "y"), (psY, "y"), (psY, "y"),
                             (psS, "s3"), (psS, "s3"), (psA, "acc")][ec]
                opst.append(pool.tile([128, 512], F32, tag=tag,
                                      name=f"opse_{ec}"))
            osb6e = outs.tile([128, CC, 512], F16, tag="osb",
                              name="osb_epi")
            for ec in range(CC):
                for c0 in (0, 256):
                    nc.tensor.matmul(
                        opst[ec][:, c0:c0 + 256],
                        lhsT=wos01[:, ec * 128:(ec + 1) * 128],
                        rhs=ynA[:, qs + c0:qs + c0 + 256],
                        start=(c0 == 0), stop=False, skip_group_check=True,
                    )
                for c0 in (0, 256):
                    nc.tensor.matmul(
                        opst[ec][:, c0:c0 + 256],
                        lhsT=wos2[:, ec * 128:(ec + 1) * 128],
                        rhs=ynB[:, qs + c0:qs + c0 + 256],
                        start=False, stop=True, skip_group_check=True,
                    )
                nc.scalar.copy(out=osb6e[:, ec, :], in_=opst[ec])
                if ec % 2 == 1:
                    q = nc.sync if ec == 1 else nc.scalar
                    q.dma_start(
                        out=outT[128 * (ec - 1):128 * (ec + 1),
                                 qs:qs + 512].rearrange(
                            "(e p) c -> p e c", e=2),
                        in_=osb6e[:, ec - 1:ec + 1, :],
                    )
    return nc


def _proj_qk_group(nc, st, w, fc, acc="acc"):
    """q/k projection for (window w, fc group): 9 DoubleRow matmuls
    (3 residual terms x 3 paired contraction chunks)."""
    ts_ = w * 512
    pool = st["psY"] if acc == "y" else st["psA"]
    ps = pool.tile([128, 512], F32, tag=acc, name=f"ps_f{fc}_t{w}")
    terms = [(st["wqhs"], st["xhs"]), (st["wqls"], st["xhs"]),
             (st["wqhs"], st["xls"])]
    for ti, (wt, xt) in enumerate(terms):
        for c0 in (0, 2, 4):
            nc.tensor.matmul(
                ps,
                lhsT=wt[:, c0:c0 + 2, fc, :],
                rhs=xt[:, c0:c0 + 2, ts_:ts_ + 512],
                start=(ti == 0 and c0 == 0),
                stop=(ti == 2 and c0 == 4),
                perf_mode=DR,
            )
    _evac_qk(nc, st, w, fc, ps)


def _evac_qk(nc, st, w, fc, ps, act=False, split=True):
    """Evacuate one fc group's psum to the flat fp8 tile (bias add for the
    q halves; the k bias columns are zero), then one SBUF->SBUF DMA to the
    [32,2,slot,512] DoubleRow layout (d = 2p+i interleave)."""
    ts_ = w * 512
    if act:
        nc.scalar.activation(
            out=st["QKF"][:, fc, ts_:ts_ + 512], in_=ps,
            func=mybir.ActivationFunctionType.Identity,
            bias=st["bqks"][:, fc:fc + 1])
    else:
        nc.vector.tensor_scalar(
            out=st["QKF"][:, fc, ts_:ts_ + 512], in0=ps,
            scalar1=st["bqks"][:, fc:fc + 1], scalar2=None,
            op0=mybir.AluOpType.add)
    if split:
        _split_qk(nc, st, w, fc)


def _split_qk(nc, st, w, fc, queue=None):
    ts_ = w * 512
    q = queue if queue is not None else (nc.gpsimd if w == 0 else nc.sync)
    q.dma_start(
        out=st["QKD"][:, fc, :, :, ts_:ts_ + 512],
        in_=st["QKF"][:, fc, ts_:ts_ + 512],
    )


def _proj_v_chunk(nc, st, tc_i, acc="acc"):
    """token-major V projection for one 128-token chunk: 9 DR matmuls."""
    pool = {"y": st["psY"], "s3": st["psS"],
            "acc": st["psA"]}[acc]
    psv = pool.tile([128, 512], F32, tag=acc, name=f"psv_{tc_i}")
    t0 = tc_i * 128
    terms = [(st["xhs"], st["wvhs"]), (st["xhs"], st["wvls"]),
             (st["xls"], st["wvhs"])]
    for ti, (xt, wt) in enumerate(terms):
        for c0 in (0, 2, 4):
            nc.tensor.matmul(
                psv[:, 0:HPC * HD],
                lhsT=xt[:, c0:c0 + 2, t0:t0 + 128],
                rhs=wt[:, c0:c0 + 2, :],
                start=(ti == 0 and c0 == 0),
                stop=(ti == 2 and c0 == 4),
                perf_mode=DR,
            )
    nc.vector.tensor_scalar(
        out=st["vT3"][:, tc_i, :, 0:HD],
        in0=psv[:, 0:HPC * HD].rearrange("p (h d) -> p h d", h=HPC),
        scalar1=1.0 / WSC, scalar2=None, op0=mybir.AluOpType.mult,
    )


def _outproj_group(nc, st, w, ec):
    qs = w * 512
    if ec == 0:
        st["osb6"] = st["outs"].tile([128, CC, 512], F16, tag="osb",
                                     name=f"osb_q{w}")
    ops = st["psA"].tile([128, 512], F32, tag="acc", name=f"ops_e{ec}_q{w}")
    nc.tensor.matmul(
        ops,
        lhsT=st["wos01"][:, ec * 128:(ec + 1) * 128],
        rhs=st["ynA"][:, qs:qs + 512],
        start=True, stop=False,
    )
    nc.tensor.matmul(
        ops,
        lhsT=st["wos2"][:, ec * 128:(ec + 1) * 128],
        rhs=st["ynB"][:, qs:qs + 512],
        start=False, stop=True,
    )
    nc.vector.tensor_copy(out=st["osb6"][:, ec, :], in_=ops)
    if ec == CC - 1:
        nc.sync.dma_start(
            out=st["outT"][:, qs:qs + 512].rearrange(
                "(e p) c -> p e c", e=CC),
            in_=st["osb6"],
        )


def _attn_window(nc, st, w):
    """Attention for q-window w across the 3 head streams, with background
    PE work (next-window projection, previous-window outproj) interleaved
    between score/PV rounds."""
    qs = w * 512
    nchunks = 4 * (w + 1)
    # below-diagonal chunks first (descending, so the first PV write is the
    # full column range), diagonal chunks last: the window's own K-side
    # projection (fc2) and V chunks then slide into this window's early
    # rounds instead of crowding the previous one.
    kc_order = list(range(4 * w))[::-1] + list(range(4 * w, nchunks))
    batches = [kc_order[i:i + EXP_BATCH]
               for i in range(0, nchunks, EXP_BATCH)]
    n_diag_batches = 2

    # bg_must: work that must land before this window's diagonal rounds.
    # bg_opt: deferrable work (previous window's normalize phase B and
    # outproj, next window's Q-side projection).
    bg_must = []
    if w == 0:
        for j in (2, 3):
            bg_must.append(lambda j=j: _proj_v_chunk(nc, st, j))
    else:
        bg_must.append(lambda: _proj_qk_group(nc, st, w, 2))
        for j in range(4):
            bg_must.append(lambda j=j: _proj_v_chunk(nc, st, 4 * w + j))
    # issue the previous window's normalize phase B immediately: it only
    # uses DVE/Pool, and it must release the y-PSUM slots before this
    # window's first (pipelined) PV round claims them.
    for s_, w_ in st.pop("pending_norm", []):
        _normalize_b(nc, st, s_, w_)
    bg_opt = []
    if w + 1 < TW:
        for fc in (0, 1):
            bg_opt.append(lambda fc=fc: _proj_qk_group(nc, st, w + 1, fc))
    if w >= 1:
        for ec in range(CC):
            bg_opt.append(lambda ec=ec: _outproj_group(nc, st, w - 1, ec))

    def bg_slot(diag_phase=False):
        if bg_must:
            bg_must.pop(0)()
        elif bg_opt:
            # hold back two opt groups to feed the PE during the
            # Act-bound diagonal rounds
            if diag_phase or len(bg_opt) > 2 or w == 0:
                bg_opt.pop(0)()

    bg = bg_opt  # leftover drain at window end uses the opt queue

    streams = [
        {"h": 0, "yn_ap": lambda q, n: st["ynA"][0:HD, q:q + n]},
        {"h": 1, "yn_ap": lambda q, n: st["ynA"][HD:128, q:q + n]},
        {"h": 2, "yn_ap": lambda q, n: st["ynB"][0:HD, q:q + n]},
    ]
    for s in streams:
        s["y"] = st["psY"].tile([128, 512], F32, tag="y",
                                name=f"y_h{s['h']}_q{w}")

    QKD = st["QKD"]

    def do_scores(s, bi, kcs):
        """scores + exp for one stream/batch; returns the et tile."""
        nb = len(kcs)
        h = s["h"]
        schrau = (SCHRAU and w == TW - 1 and h == 2
                  and all(kc < 4 * w for kc in kcs)
                  and bi >= 4)
        s_ps = st["psS"].tile([128, EXP_BATCH, 512], F32, tag="s3",
                              name=f"s_h{h}_q{w}_b{bi}")
        if schrau:
            eti = st["ets"].tile([128, EXP_BATCH, 512], mybir.dt.int16,
                                 tag="et", name=f"e_h{h}_q{w}_b{bi}")
            et = eti.bitcast(mybir.dt.bfloat16)
        else:
            et = st["ets"].tile([128, EXP_BATCH, 512], F16, tag="et",
                                name=f"e_h{h}_q{w}_b{bi}")
        js = [max(0, kc - 4 * w) for kc in kcs]
        jw = [min(js)] * nb
        kf, ks = KPOS[h]
        qf, qsl = QPOS[h]
        for i in range(nb):
            kc = kcs[i]
            j = jw[i]
            nc.tensor.matmul(
                s_ps[:, i, 128 * j:512],
                lhsT=QKD[:, kf, :, ks, kc * 128:(kc + 1) * 128],
                rhs=QKD[:, qf, :, qsl, qs + 128 * j:qs + 512],
                start=True, stop=True,
                perf_mode=DR,
            )
        if schrau:
            nc.vector.tensor_scalar(
                out=eti[:, 0:nb, :], in0=s_ps[:, 0:nb, :],
                scalar1=SCHRAU_A * ESCALE, scalar2=SCHRAU_B,
                op0=mybir.AluOpType.mult, op1=mybir.AluOpType.add)
            return et
        jm = min(js)
        nc.scalar.activation(
            out=et[:, 0:nb, 128 * jm:512],
            in_=s_ps[:, 0:nb, 128 * jm:512],
            func=mybir.ActivationFunctionType.Exp, scale=ESCALE,
        )
        for i in range(nb):
            j = kcs[i] - 4 * w
            if j < 0:
                continue
            nc.gpsimd.tensor_mul(
                out=et[:, i, 128 * j:128 * (j + 1)],
                in0=et[:, i, 128 * j:128 * (j + 1)],
                in1=st["mask"],
            )
        return et

    def do_pv(s, bi, kcs, et):
        for i in range(len(kcs)):
            kc = kcs[i]
            j = max(0, kc - 4 * w)
            idx = bi * EXP_BATCH + i
            nc.tensor.matmul(
                s["y"][0:HD + 1, 128 * j:512],
                lhsT=st["vT3"][:, kc, s["h"], 0:HD + 1],
                rhs=et[:, i, 128 * j:512],
                start=(idx == 0), stop=(idx == nchunks - 1),
                skip_group_check=True,
            )

    # software-pipelined rounds: batch b's scores/exp run interleaved with
    # batch b-1's PV, so the PE never sits behind an exp chain even when
    # the bg queues run dry. The ets pool (6 bufs) holds exactly two
    # rounds x 3 streams.
    prev = None  # (bi, kcs, {h: et})
    for bi, kcs in enumerate(batches):
        diag_phase = bi >= len(batches) - n_diag_batches
        if bi == len(batches) - n_diag_batches:
            while bg_must:
                bg_must.pop(0)()
        ebt = {}
        for si, s in enumerate(streams):
            if si == 2:
                bg_slot(diag_phase)
            ebt[s["h"]] = do_scores(s, bi, kcs)
            if prev is not None:
                do_pv(s, prev[0], prev[1], prev[2][s["h"]])
                if w == TW - 1 and prev[0] == len(batches) - 2:
                    _normalize_half(nc, st, s, w, 0)
        prev = (bi, kcs, ebt)
    # flush: last batch's PV (+ normalize) for each stream. On the last
    # window the epilogue output projection is woven in per column half:
    # phase A (cols 0:256) runs as soon as the half-0 normalizes are done,
    # each psY-borrowing accumulator right after its stream's y retires.
    for si, s in enumerate(streams):
        if si == 2 and (bg_must or bg):
            (bg_must or bg).pop(0)()
        do_pv(s, prev[0], prev[1], prev[2][s["h"]])
        if w == TW - 1:
            _normalize_half(nc, st, s, w, 256)
        else:
            _normalize_a(nc, st, s, w)
    if w != TW - 1:
        st["pending_norm"] = [(s, w) for s in streams]

    # leftover background groups
    while bg:
        bg.pop(0)()


def _epi_start(nc, st, w):
    """Allocate the epilogue accumulators/staging: ec0/ec1 borrow psS,
    ec2 borrows psA, ec3..5 borrow the three psY slots (which free in
    stream flush order h0, h1, h2)."""
    pools = [(st["psS"], "s3"), (st["psS"], "s3"), (st["psA"], "acc"),
             (st["psY"], "y"), (st["psY"], "y"), (st["psY"], "y")]
    st["opst"] = [pool.tile([128, 512], F32, tag=tag, name=f"opse_{ec}")
                  for ec, (pool, tag) in enumerate(pools)]
    st["osb6e"] = st["outs"].tile([128, CC, 512], F16, tag="osb",
                                  name="osb_epi")


def _epi_a(nc, st, w, ecs):
    qs = w * 512
    for ec in ecs:
        nc.tensor.matmul(
            st["opst"][ec][:, 0:256],
            lhsT=st["wos01"][:, ec * 128:(ec + 1) * 128],
            rhs=st["ynA"][:, qs:qs + 256],
            start=True, stop=False, skip_group_check=True,
        )
        nc.tensor.matmul(
            st["opst"][ec][:, 0:256],
            lhsT=st["wos2"][:, ec * 128:(ec + 1) * 128],
            rhs=st["ynB"][:, qs:qs + 256],
            start=False, stop=True, skip_group_check=True,
        )


def _epi_evac(nc, st, w, c0):
    """evacuate + store one column half for all ec (Act copies: DVE is
    draining the normalize chains that gate these matmuls)."""
    qs = w * 512
    osb = st["osb6e"]
    for ec in range(CC):
        nc.scalar.copy(out=osb[:, ec, c0:c0 + 256],
                       in_=st["opst"][ec][:, c0:c0 + 256])
        if ec % 2 == 1:
            nc.sync.dma_start(
                out=st["outT"][128 * (ec - 1):128 * (ec + 1),
                               qs + c0:qs + c0 + 256].rearrange(
                    "(e p) c -> p e c", e=2),
                in_=osb[:, ec - 1:ec + 1, c0:c0 + 256],
            )


def _epi_b(nc, st, w):
    qs = w * 512
    for ec in range(CC):
        nc.tensor.matmul(
            st["opst"][ec][:, 256:512],
            lhsT=st["wos01"][:, ec * 128:(ec + 1) * 128],
            rhs=st["ynA"][:, qs + 256:qs + 512],
            start=True, stop=False, skip_group_check=True,
        )
        nc.tensor.matmul(
            st["opst"][ec][:, 256:512],
            lhsT=st["wos2"][:, ec * 128:(ec + 1) * 128],
            rhs=st["ynB"][:, qs + 256:qs + 512],
            start=False, stop=True, skip_group_check=True,
        )


def _normalize_half(nc, st, s, w, c0):
    """full normalize chain for one 256-wide column half (last window)."""
    qs = w * 512
    h = s["h"]
    y = s["y"]
    if c0 == 0:
        s["sc"] = st["scr"].tile([128, 512], F32, tag="sc",
                                 name=f"sc_h{h}_q{w}")
        s["rbt"] = st["scr"].tile([HD, 512], F32, tag="rbs",
                                  name=f"rb_h{h}_q{w}")
    sc, rb = s["sc"], s["rbt"]
    nc.vector.reciprocal(out=sc[0:1, c0:c0 + 256],
                         in_=y[HD:HD + 1, c0:c0 + 256])
    nc.gpsimd.partition_broadcast(rb[:, c0:c0 + 256], sc[0:1, c0:c0 + 256])
    nc.vector.tensor_mul(
        out=s["yn_ap"](qs + c0, 256),
        in0=y[0:HD, c0:c0 + 256], in1=rb[:, c0:c0 + 256],
    )


def _normalize_a(nc, st, s, w):
    """reciprocal of the sumexp row."""
    h = s["h"]
    y = s["y"]
    sc = st["scr"].tile([128, 512], F32, tag="sc", name=f"sc_h{h}_q{w}")
    s["sc"] = sc
    nc.vector.reciprocal(out=sc[0:1, 0:512], in_=y[HD:HD + 1, 0:512])


def _normalize_b(nc, st, s, w):
    """broadcast 1/sumexp to 64 partitions on GPSIMD, then
    y[0:64] * rb -> yn (Act copy + Pool multiply; see _normalize_half)."""
    qs = w * 512
    h = s["h"]
    y = s["y"]
    sc = s["sc"]
    rb = st["scr"].tile([HD, 512], F32, tag="rbs", name=f"rb_h{h}_q{w}")
    nc.gpsimd.partition_broadcast(rb[:, 0:512], sc[0:1, 0:512])
    nc.vector.tensor_mul(
        out=s["yn_ap"](qs, 512),
        in0=y[0:HD, 0:512], in1=rb[:, 0:512],
    )


def _prep_core_inputs(c, x, w_qkv, b_qkv, w_out):
    b = c // CPB
    g = c % CPB
    hs = [HPC * g + i for i in range(HPC)]

    def q8(a):
        return a.astype(E4NP)

    # fc column orders (interleaved for the d=2p+i split DMA):
    # fc0: [q_h0 d0, q_h1 d0, q_h0 d1, q_h1 d1, ...]
    # fc1: [q_h2 d0, k_h2 d0, q_h2 d1, k_h2 d1, ...]
    # fc2: [k_h0 d0, k_h1 d0, ...]
    d = np.arange(HD)
    qc = [h * HD + d for h in hs]                    # q cols per head
    kc_ = [D + h * HD + d for h in hs]               # k cols per head
    vc = [2 * D + h * HD + d for h in hs]

    fc0 = np.stack([qc[0], qc[1]], axis=1).reshape(-1)
    fc1 = np.stack([qc[2], kc_[2]], axis=1).reshape(-1)
    fc2 = np.stack([kc_[0], kc_[1]], axis=1).reshape(-1)
    cols = np.concatenate([fc0, fc1, fc2])
    vcols = np.concatenate(vc)

    wS = (w_qkv[:, cols] * WSC).astype(np.float32)
    wh = q8(wS)
    wl = q8(wS - wh.astype(np.float32))
    wvS = (w_qkv[:, vcols] * WSC).astype(np.float32)
    wvh_ = q8(wvS)
    wvl_ = q8(wvS - wvh_.astype(np.float32))

    xT = np.ascontiguousarray(x[b].T).astype(np.float32)
    xh_ = q8(xT)
    xl_ = q8(xT - xh_.astype(np.float32))

    # bias columns: fc0 = bq(h0|h1 interleaved)*WSC; fc1 = bq_h2 at even
    # partitions, 0 at odd (k_h2: bias dropped); fc2 = 0
    bq = b_qkv[:D]
    bcol = np.zeros((128, 3), dtype=np.float32)
    bcol[:, 0] = np.stack([bq[qc[0]], bq[qc[1]]], axis=1).reshape(-1) * WSC
    b1 = np.zeros(128, dtype=np.float32)
    b1[0::2] = bq[qc[2]] * WSC
    bcol[:, 1] = b1

    return {
        "xh": np.ascontiguousarray(xh_.reshape(CC, 128, T)),
        "xl": np.ascontiguousarray(xl_.reshape(CC, 128, T)),
        "wqh": np.ascontiguousarray(wh.reshape(CC, 128, 3, 128)),
        "wql": np.ascontiguousarray(wl.reshape(CC, 128, 3, 128)),
        "bqk": bcol,
        "wvh": np.ascontiguousarray(wvh_.reshape(CC, 128, HPC * HD)),
        "wvl": np.ascontiguousarray(wvl_.reshape(CC, 128, HPC * HD)),
        "wo01": np.ascontiguousarray(
            w_out[192 * g:192 * g + 128, :].astype(np.float16)),
        "wo2": np.ascontiguousarray(
            w_out[192 * g + 128:192 * g + 192, :].astype(np.float16)),
    }


_NC_CACHE = {}


def get_nc():
    if "nc" not in _NC_CACHE:
        nc = build_bass()
        nc.finalize()
        _NC_CACHE["nc"] = nc
    return _NC_CACHE["nc"]


def kernel(x, w_qkv, b_qkv, w_out, b_out, _run_kwargs=None):
    x = np.asarray(x, dtype=np.float32)
    w_qkv = np.asarray(w_qkv, dtype=np.float32)
    b_qkv = np.asarray(b_qkv, dtype=np.float32)
    w_out = np.asarray(w_out, dtype=np.float32)
    b_out = np.asarray(b_out, dtype=np.float32)

    nc = get_nc()
    in_maps = [_prep_core_inputs(c, x, w_qkv, b_qkv, w_out)
               for c in range(NCORES)]
    kwargs = dict(_run_kwargs or {})
    res = run_bass_kernel_spmd(nc, in_maps, core_ids=list(range(NCORES)),
                               **kwargs)
    if kwargs:
        _NC_CACHE["last_results"] = res

    bv_corr = b_qkv[2 * D:3 * D] @ w_out  # [D]
    out = np.zeros((B, T, D), dtype=np.float32)
    for b in range(B):
        acc = np.zeros((T, D), dtype=np.float32)
        for g in range(CPB):
            acc += np.asarray(res.results[b * CPB + g]["outT"]
                              ).astype(np.float32).T
        out[b] = acc + (b_out + bv_corr)[None, :]
    return out


if __name__ == "__main__":
    nc = build_bass()
    print("built OK")


# revision 73
# speedup vs baseline: 1.2336x; 1.0540x over previous
"""Causal self-attention (B=2, T=2048, D=768, H=12) on 8 TRN2 NeuronCores.

Sharding: tensor-parallel over (batch, head) pairs; 3 heads per core, one
batch per 4-core group; the host sums the 4 partial outputs per batch and
adds b_out (+ the v-bias correction through w_out).

fp8 strategy (the PE cost model charges output-columns x cycles/row, with
fp8e4+DoubleRow at 0.5 cycles/row and contraction depth free):
- QKV / V projections run as 3-term residual fp8 DoubleRow matmuls:
  x = xh + xl (hi/lo fp8), w*64 = wh + wl, and
  psum = xh@wh + xh@wl + xl@wh (all same scale, lo*lo dropped, ~0.1% err).
  DoubleRow pairs adjacent contraction chunks ([128,2,*] APs), so the
  6-chunk contraction is 9 DR matmuls at half rate (vs 6 full-rate fp16).
- Scores run as fp8 DoubleRow with the head dim split 2x32: Q/K are
  evacuated to a flat fp8 tile (Q gets +bias*64; K's bias is dropped -- it
  is constant along the softmax axis, hence exactly softmax-invariant),
  then one SBUF->SBUF DMA per fc-group splits [128,512] into the
  [32,2,slot,512] layout (d = 2p+i interleave, heads pre-interleaved in
  the weight column order by the host).
- exp scale absorbs the 64*64 weight scaling: exp(s * SCALE/4096).
- PV and the output projection stay fp16 (attention-weight and value
  precision bound the error budget; measured ~1e-2 end-to-end).

Per 512-token q-window, the scores->exp->PV chain runs in "rounds"
of 2-k-chunk batches across the 3 head streams; below-diagonal chunks run
first, and background PE work (the window's K-side projection + V chunks,
the next window's Q-side projection, the previous window's output
projection, deferred normalize phases) is interleaved between rounds so
the PE keeps running while ScalarE exp latency drains. The softmax
normalizer uses a reciprocal on DVE plus a GPSIMD partition_broadcast.
A warm-up run of free N=1 matmuls starts the PE p-state ramp during the
initial DMA wait.

Constraints learned on real silicon (CoreSim does not model them):
GPSIMD cannot access PSUM at all; tensor_tensor cannot read two PSUM
operands; PSUM matmul outputs must start at partition 0.
"""

import numpy as np
import ml_dtypes

import concourse.bass as bass
import concourse.bacc as bacc
import concourse.mybir as mybir
import concourse.tile as tile
from concourse import library_config
from concourse.masks import make_upper_triangular
from concourse.bass_utils import run_bass_kernel_spmd

B, T, D, H, HD = 2, 2048, 768, 12, 64
NCORES = 8
HPC = 3            # heads per core
CPB = NCORES // B  # cores per batch = 4
CC = D // 128      # d_model chunks of 128 = 6
TW = T // 512      # token windows of 512 = 4
KC = T // 128      # k chunks of 128 = 16
SCALE = 1.0 / float(np.sqrt(HD))
WSC = 64.0         # fp8 weight pre-scale (folded into the exp scale / evacs)
ESCALE = SCALE / (WSC * WSC)   # exp scale for raw fp8-score psums

F8 = mybir.dt.float8e4
F16 = mybir.dt.float16
F32 = mybir.dt.float32
E4NP = ml_dtypes.float8_e4m3fn
DR = mybir.MatmulPerfMode.DoubleRow

EXP_BATCH = 2  # k-chunks per exp call / per s-tile (PSUM tile = 2 banks)

# QKD layout [32, fc, i, slot, T] (i = d-pair index, slot = head lane):
# per head h: q at (fcg, sl) and k at (fcg, sl) below
QPOS = ((0, 0), (0, 1), (1, 0))   # h0, h1, h2
KPOS = ((2, 0), (2, 1), (1, 1))

# Schraudolph fast-exp on DVE for stream h2's below-diagonal batches in
# the last window (relieves the ScalarE bottleneck there). exp(x) ~ bf16
# bitcast of int16(A*x + B); ~2.4% RMS approximation error on ~11% of the
# attention weights.
SCHRAU = True
SCHRAU_A = 128.0 / np.log(2.0)
SCHRAU_B = float(127 << 7) - 7.5


def build_bass():
    nc = bacc.Bacc(None, target_bir_lowering=False)

    xh = nc.dram_tensor("xh", [CC, 128, T], F8, kind="ExternalInput")
    xl = nc.dram_tensor("xl", [CC, 128, T], F8, kind="ExternalInput")
    wqh = nc.dram_tensor("wqh", [CC, 128, 3, 128], F8, kind="ExternalInput")
    bqk = nc.dram_tensor("bqk", [128, 3], F32, kind="ExternalInput")
    wvh = nc.dram_tensor("wvh", [CC, 128, HPC * HD], F8, kind="ExternalInput")
    wvl = nc.dram_tensor("wvl", [CC, 128, HPC * HD], F8, kind="ExternalInput")
    wo01d = nc.dram_tensor("wo01", [128, D], F16, kind="ExternalInput")
    wo2d = nc.dram_tensor("wo2", [HD, D], F16, kind="ExternalInput")
    outT = nc.dram_tensor("outT", [D, T], F16, kind="ExternalOutput")

    with tile.TileContext(nc) as tc:
        with (
            tc.tile_pool(name="big", bufs=1) as big,
            tc.tile_pool(name="ets", bufs=6) as ets,
            tc.tile_pool(name="scr", bufs=3) as scr,
            tc.tile_pool(name="outs", bufs=4) as outs,
            tc.tile_pool(name="psS", bufs=2, space="PSUM") as psS,
            tc.tile_pool(name="psY", bufs=3, space="PSUM") as psY,
            tc.tile_pool(name="psA", bufs=1, space="PSUM") as psA,
        ):
            # ---- SBUF persistent tiles ----
            wqhs = big.tile([128, CC, 3, 128], F8, tag="wqh")
            wvhs = big.tile([128, CC, HPC * HD], F8, tag="wvh")
            wvls = big.tile([128, CC, HPC * HD], F8, tag="wvl")
            xhs = big.tile([128, CC, T], F8, tag="xh")
            xls = big.tile([128, CC, T], F8, tag="xl")
            bqks = big.tile([128, 3], F32, tag="bqk")
            wos01 = big.tile([128, D], F16, tag="wo01")
            wos2 = big.tile([HD, D], F16, tag="wo2")
            QKF = big.tile([128, 3, T], F8, tag="QKF")      # flat staging
            QKD = big.tile([32, 3, 2, 2, T], F8, tag="QKD")  # DR split layout
            # token-major V (+ ones column at 64): [128, kc, h, 66]
            vT3 = big.tile([128, KC, HPC, 66], F16, tag="vT3")
            ynA = big.tile([128, T], F16, tag="ynA")
            ynB = big.tile([HD, T], F16, tag="ynB")
            mask_tri = big.tile([128, 128], F16, tag="mask")

            # PE p-state warm-up: a dense run of ~free N=1 matmuls starts
            # the tensor engine's ramp clock during the input-DMA wait so
            # the real matmuls reach full clock ~2us earlier; the dummy Exp
            # pulls the activation-table load (1.3us) off the first real
            # exp's critical path.
            wtiny = big.tile([1, 8], F16, tag="wtiny")
            nc.vector.memset(wtiny, 0.5)
            nc.scalar.activation(out=wtiny[0:1, 4:8], in_=wtiny[0:1, 0:4],
                                 func=mybir.ActivationFunctionType.Exp)
            s_warm = psS.tile([128, EXP_BATCH, 512], F32, tag="s3",
                              name="s_warm")
            warm_cols = 1024
            for i in range(120):
                j = i % warm_cols
                nc.tensor.matmul(
                    s_warm[0:1, j // 512, j % 512:j % 512 + 1],
                    lhsT=wtiny[0:1, 0:1], rhs=wtiny[0:1, 0:1],
                    start=True, stop=True, skip_group_check=True)

            # ---- input DMAs: window-0 criticals first. Weights go through
            # the Pool SWDGE queue while x window-0 chunks go through the
            # SP HWDGE queue -- two parallel descriptor-generation paths.
            nc.gpsimd.dma_start(
                out=wqhs[:, 0:2], in_=wqh[0:2].rearrange("c p f k -> p c f k"))
            nc.gpsimd.dma_start(out=xhs[:, 4, 0:512], in_=xh[4, :, 0:512])
            nc.gpsimd.dma_start(out=xhs[:, 5, 0:512], in_=xh[5, :, 0:512])
            nc.gpsimd.dma_start(
                out=wqhs[:, 4:6], in_=wqh[4:6].rearrange("c p f k -> p c f k"))
            # the GPSIMD ucode library carrying partition_broadcast loads
            # after the startup DMAs so it doesn't gate them
            nc.gpsimd.load_library(library_config.proxy)
            # SP queue: x hi window-0 + the q/k weight pair not on Pool
            nc.sync.dma_start(
                out=xhs[:, 0:2, 0:512],
                in_=xh[0:2, :, 0:512].rearrange("c p t -> p c t"))
            nc.sync.dma_start(
                out=wqhs[:, 2:4], in_=wqh[2:4].rearrange("c p f k -> p c f k"))
            nc.sync.dma_start(
                out=xhs[:, 2:4, 0:512],
                in_=xh[2:4, :, 0:512].rearrange("c p t -> p c t"))
            nc.sync.dma_start(out=bqks, in_=bqk[:, :])
            # Act queue (idle at startup): lo-residual inputs; V weights ride
            # the Pool SWDGE queue
            for c0 in (0, 2, 4):
                nc.scalar.dma_start(
                    out=xls[:, c0:c0 + 2, 0:512],
                    in_=xl[c0:c0 + 2, :, 0:512].rearrange("c p t -> p c t"))
            nc.gpsimd.dma_start(
                out=wvhs, in_=wvh.rearrange("c p f -> p c f"))
            nc.gpsimd.dma_start(
                out=wvls, in_=wvl.rearrange("c p f -> p c f"))
            # x tails: window-1 token range first so window-1 projections
            # (window-0 bg slots) aren't input-starved; single big DMAs to
            # keep the SP issue queue short
            nc.sync.dma_start(
                out=xhs[:, :, 512:1024],
                in_=xh[:, :, 512:1024].rearrange("c p t -> p c t"))
            nc.sync.dma_start(
                out=xls[:, :, 512:1024],
                in_=xl[:, :, 512:1024].rearrange("c p t -> p c t"))


            # ---- constants ----
            make_upper_triangular(nc, mask_tri, val=1.0, diag=True)
            for h in range(HPC):
                nc.gpsimd.memset(vT3[:, :, h, HD:HD + 1], 1.0)

            st = {
                "pending_norm": [],
                "wqhs": wqhs, "wvhs": wvhs, "wvls": wvls,
                "xhs": xhs, "xls": xls, "bqks": bqks,
                "wos01": wos01, "wos2": wos2, "QKF": QKF, "QKD": QKD,
                "vT3": vT3, "ynA": ynA, "ynB": ynB,
                "mask": mask_tri,
                "psS": psS, "psY": psY, "psA": psA,
                "ets": ets, "scr": scr, "outs": outs, "outT": outT,
            }

            # prologue: window-0 q/k projection. 3 psY accumulators, one per
            # fc group; term-major (hh, hl, lh) so each arriving input is
            # consumed immediately; V chunks 0/1 here, 2/3 ride window 0's
            # bg queue.
            paccs = [psY.tile([128, 512], F32, tag="y", name=f"pacc_f{fc}")
                     for fc in range(3)]
            # q/k: 2-term residual (x_hi + x_lo, single-fp8 weights); the
            # score-path tolerates the w-quantization since q/k are fp8
            # re-quantized at the evac anyway
            terms = [(wqhs, xhs), (wqhs, xls)]
            for ti, (wt, xt) in enumerate(terms):
                for c0 in (0, 2, 4):
                    for fc in range(3):
                        nc.tensor.matmul(
                            paccs[fc],
                            lhsT=wt[:, c0:c0 + 2, fc, :],
                            rhs=xt[:, c0:c0 + 2, 0:512],
                            start=(ti == 0 and c0 == 0),
                            stop=(ti == 1 and c0 == 4),
                            perf_mode=DR,
                        )
            # spread window-0's three split DMAs across three queues so
            # their descriptor generations run in parallel
            _evac_qk(nc, st, 0, 0, paccs[0], act=False, split=False)
            _evac_qk(nc, st, 0, 1, paccs[1], act=True, split=False)
            _evac_qk(nc, st, 0, 2, paccs[2], act=True, split=False)
            _split_qk(nc, st, 0, 0, queue=nc.sync)
            _split_qk(nc, st, 0, 2, queue=nc.scalar)
            _split_qk(nc, st, 0, 1, queue=nc.gpsimd)
            _proj_v_chunk(nc, st, 0)
            _proj_v_chunk(nc, st, 1, acc="s3")
            nc.sync.dma_start(
                out=xhs[:, :, 1024:T],
                in_=xh[:, :, 1024:T].rearrange("c p t -> p c t"))
            nc.sync.dma_start(out=wos01, in_=wo01d[:, :])
            nc.sync.dma_start(out=wos2, in_=wo2d[:, :])
            nc.sync.dma_start(
                out=xls[:, :, 1024:T],
                in_=xl[:, :, 1024:T].rearrange("c p t -> p c t"))

            for w in range(TW):
                _attn_window(nc, st, w)

            # epilogue: output projection for the last window
            qs = (TW - 1) * 512
            opst = []
            for ec in range(CC):
                pool, tag = [(psY, "y"), (psY, "y"), (psY, "y"),
                             (psS, "s3"), (psS, "s3"), (psA, "acc")][ec]
                opst.append(pool.tile([128, 512], F32, tag=tag,
                                      name=f"opse_{ec}"))
            osb6e = outs.tile([128, CC, 512], F16, tag="osb",
                              name="osb_epi")
            for ec in range(CC):
                for c0 in (0, 256):
                    nc.tensor.matmul(
                        opst[ec][:, c0:c0 + 256],
                        lhsT=wos01[:, ec * 128:(ec + 1) * 128],
                        rhs=ynA[:, qs + c0:qs + c0 + 256],
                        start=(c0 == 0), stop=False, skip_group_check=True,
                    )
                for c0 in (0, 256):
                    nc.tensor.matmul(
                        opst[ec][:, c0:c0 + 256],
                        lhsT=wos2[:, ec * 128:(ec + 1) * 128],
                        rhs=ynB[:, qs + c0:qs + c0 + 256],
                        start=False, stop=True, skip_group_check=True,
                    )
                if ec % 2 == 0:
                    nc.scalar.copy(out=osb6e[:, ec, :], in_=opst[ec])
                else:
                    nc.vector.tensor_copy(out=osb6e[:, ec, :],
                                          in_=opst[ec])
                if ec % 2 == 1:
                    q = nc.sync if ec == 1 else nc.scalar
                    q.dma_start(
                        out=outT[128 * (ec - 1):128 * (ec + 1),
                                 qs:qs + 512].rearrange(
                            "(e p) c -> p e c", e=2),
                        in_=osb6e[:, ec - 1:ec + 1, :],
                    )
    return nc


def _proj_qk_group(nc, st, w, fc, acc="acc"):
    """q/k projection for (window w, fc group): 9 DoubleRow matmuls
    (3 residual terms x 3 paired contraction chunks)."""
    ts_ = w * 512
    pool = st["psY"] if acc == "y" else st["psA"]
    ps = pool.tile([128, 512], F32, tag=acc, name=f"ps_f{fc}_t{w}")
    terms = [(st["wqhs"], st["xhs"]), (st["wqhs"], st["xls"])]
    for ti, (wt, xt) in enumerate(terms):
        for c0 in (0, 2, 4):
            nc.tensor.matmul(
                ps,
                lhsT=wt[:, c0:c0 + 2, fc, :],
                rhs=xt[:, c0:c0 + 2, ts_:ts_ + 512],
                start=(ti == 0 and c0 == 0),
                stop=(ti == 1 and c0 == 4),
                perf_mode=DR,
            )
    _evac_qk(nc, st, w, fc, ps)


def _evac_qk(nc, st, w, fc, ps, act=False, split=True):
    """Evacuate one fc group's psum to the flat fp8 tile (bias add for the
    q halves; the k bias columns are zero), then one SBUF->SBUF DMA to the
    [32,2,slot,512] DoubleRow layout (d = 2p+i interleave)."""
    ts_ = w * 512
    if act:
        nc.scalar.activation(
            out=st["QKF"][:, fc, ts_:ts_ + 512], in_=ps,
            func=mybir.ActivationFunctionType.Identity,
            bias=st["bqks"][:, fc:fc + 1])
    else:
        nc.vector.tensor_scalar(
            out=st["QKF"][:, fc, ts_:ts_ + 512], in0=ps,
            scalar1=st["bqks"][:, fc:fc + 1], scalar2=None,
            op0=mybir.AluOpType.add)
    if split:
        _split_qk(nc, st, w, fc)


def _split_qk(nc, st, w, fc, queue=None):
    ts_ = w * 512
    q = queue if queue is not None else (nc.gpsimd if w == 0 else nc.sync)
    q.dma_start(
        out=st["QKD"][:, fc, :, :, ts_:ts_ + 512],
        in_=st["QKF"][:, fc, ts_:ts_ + 512],
    )


def _proj_v_chunk(nc, st, tc_i, acc="acc"):
    """token-major V projection for one 128-token chunk: 9 DR matmuls."""
    pool = {"y": st["psY"], "s3": st["psS"],
            "acc": st["psA"]}[acc]
    psv = pool.tile([128, 512], F32, tag=acc, name=f"psv_{tc_i}")
    t0 = tc_i * 128
    terms = [(st["xhs"], st["wvhs"]), (st["xhs"], st["wvls"]),
             (st["xls"], st["wvhs"])]
    for ti, (xt, wt) in enumerate(terms):
        for c0 in (0, 2, 4):
            nc.tensor.matmul(
                psv[:, 0:HPC * HD],
                lhsT=xt[:, c0:c0 + 2, t0:t0 + 128],
                rhs=wt[:, c0:c0 + 2, :],
                start=(ti == 0 and c0 == 0),
                stop=(ti == 2 and c0 == 4),
                perf_mode=DR,
            )
    nc.vector.tensor_scalar(
        out=st["vT3"][:, tc_i, :, 0:HD],
        in0=psv[:, 0:HPC * HD].rearrange("p (h d) -> p h d", h=HPC),
        scalar1=1.0 / WSC, scalar2=None, op0=mybir.AluOpType.mult,
    )


def _outproj_group(nc, st, w, ec):
    qs = w * 512
    if ec == 0:
        st["osb6"] = st["outs"].tile([128, CC, 512], F16, tag="osb",
                                     name=f"osb_q{w}")
    ops = st["psA"].tile([128, 512], F32, tag="acc", name=f"ops_e{ec}_q{w}")
    nc.tensor.matmul(
        ops,
        lhsT=st["wos01"][:, ec * 128:(ec + 1) * 128],
        rhs=st["ynA"][:, qs:qs + 512],
        start=True, stop=False,
    )
    nc.tensor.matmul(
        ops,
        lhsT=st["wos2"][:, ec * 128:(ec + 1) * 128],
        rhs=st["ynB"][:, qs:qs + 512],
        start=False, stop=True,
    )
    nc.vector.tensor_copy(out=st["osb6"][:, ec, :], in_=ops)
    if ec == CC - 1:
        nc.sync.dma_start(
            out=st["outT"][:, qs:qs + 512].rearrange(
                "(e p) c -> p e c", e=CC),
            in_=st["osb6"],
        )


def _attn_window(nc, st, w):
    """Attention for q-window w across the 3 head streams, with background
    PE work (next-window projection, previous-window outproj) interleaved
    between score/PV rounds."""
    qs = w * 512
    nchunks = 4 * (w + 1)
    # below-diagonal chunks first (descending, so the first PV write is the
    # full column range), diagonal chunks last: the window's own K-side
    # projection (fc2) and V chunks then slide into this window's early
    # rounds instead of crowding the previous one.
    kc_order = list(range(4 * w))[::-1] + list(range(4 * w, nchunks))
    batches = [kc_order[i:i + EXP_BATCH]
               for i in range(0, nchunks, EXP_BATCH)]
    n_diag_batches = 2

    # bg_must: work that must land before this window's diagonal rounds.
    # bg_opt: deferrable work (previous window's normalize phase B and
    # outproj, next window's Q-side projection).
    bg_must = []
    if w == 0:
        for j in (2, 3):
            bg_must.append(lambda j=j: _proj_v_chunk(nc, st, j))
    else:
        bg_must.append(lambda: _proj_qk_group(nc, st, w, 2))
        for j in range(4):
            bg_must.append(lambda j=j: _proj_v_chunk(nc, st, 4 * w + j))
    # issue the previous window's normalize phase B immediately: it only
    # uses DVE/Pool, and it must release the y-PSUM slots before this
    # window's first (pipelined) PV round claims them.
    for s_, w_ in st.pop("pending_norm", []):
        _normalize_b(nc, st, s_, w_)
    bg_opt = []
    if w + 1 < TW:
        for fc in (0, 1):
            bg_opt.append(lambda fc=fc: _proj_qk_group(nc, st, w + 1, fc))
    if w >= 1:
        for ec in range(CC):
            bg_opt.append(lambda ec=ec: _outproj_group(nc, st, w - 1, ec))

    def bg_slot(diag_phase=False):
        if bg_must:
            bg_must.pop(0)()
        elif bg_opt:
            # hold back two opt groups to feed the PE during the
            # Act-bound diagonal rounds
            if diag_phase or len(bg_opt) > 2 or w == 0:
                bg_opt.pop(0)()

    bg = bg_opt  # leftover drain at window end uses the opt queue

    streams = [
        {"h": 0, "yn_ap": lambda q, n: st["ynA"][0:HD, q:q + n]},
        {"h": 1, "yn_ap": lambda q, n: st["ynA"][HD:128, q:q + n]},
        {"h": 2, "yn_ap": lambda q, n: st["ynB"][0:HD, q:q + n]},
    ]
    for s in streams:
        s["y"] = st["psY"].tile([128, 512], F32, tag="y",
                                name=f"y_h{s['h']}_q{w}")

    QKD = st["QKD"]

    def do_scores(s, bi, kcs):
        """scores + exp for one stream/batch; returns the et tile."""
        nb = len(kcs)
        h = s["h"]
        schrau = (SCHRAU and w == TW - 1 and h == 2
                  and all(kc < 4 * w for kc in kcs)
                  and bi >= 4)
        s_ps = st["psS"].tile([128, EXP_BATCH, 512], F32, tag="s3",
                              name=f"s_h{h}_q{w}_b{bi}")
        if schrau:
            eti = st["ets"].tile([128, EXP_BATCH, 512], mybir.dt.int16,
                                 tag="et", name=f"e_h{h}_q{w}_b{bi}")
            et = eti.bitcast(mybir.dt.bfloat16)
        else:
            et = st["ets"].tile([128, EXP_BATCH, 512], F16, tag="et",
                                name=f"e_h{h}_q{w}_b{bi}")
        js = [max(0, kc - 4 * w) for kc in kcs]
        jw = [min(js)] * nb
        kf, ks = KPOS[h]
        qf, qsl = QPOS[h]
        for i in range(nb):
            kc = kcs[i]
            j = jw[i]
            nc.tensor.matmul(
                s_ps[:, i, 128 * j:512],
                lhsT=QKD[:, kf, :, ks, kc * 128:(kc + 1) * 128],
                rhs=QKD[:, qf, :, qsl, qs + 128 * j:qs + 512],
                start=True, stop=True,
                perf_mode=DR,
            )
        if schrau:
            nc.vector.tensor_scalar(
                out=eti[:, 0:nb, :], in0=s_ps[:, 0:nb, :],
                scalar1=SCHRAU_A * ESCALE, scalar2=SCHRAU_B,
                op0=mybir.AluOpType.mult, op1=mybir.AluOpType.add)
            return et
        jm = min(js)
        nc.scalar.activation(
            out=et[:, 0:nb, 128 * jm:512],
            in_=s_ps[:, 0:nb, 128 * jm:512],
            func=mybir.ActivationFunctionType.Exp, scale=ESCALE,
        )
        for i in range(nb):
            j = kcs[i] - 4 * w
            if j < 0:
                continue
            nc.gpsimd.tensor_mul(
                out=et[:, i, 128 * j:128 * (j + 1)],
                in0=et[:, i, 128 * j:128 * (j + 1)],
                in1=st["mask"],
            )
        return et

    def do_pv(s, bi, kcs, et):
        for i in range(len(kcs)):
            kc = kcs[i]
            j = max(0, kc - 4 * w)
            idx = bi * EXP_BATCH + i
            nc.tensor.matmul(
                s["y"][0:HD + 1, 128 * j:512],
                lhsT=st["vT3"][:, kc, s["h"], 0:HD + 1],
                rhs=et[:, i, 128 * j:512],
                start=(idx == 0), stop=(idx == nchunks - 1),
                skip_group_check=True,
            )

    # software-pipelined rounds: batch b's scores/exp run interleaved with
    # batch b-1's PV, so the PE never sits behind an exp chain even when
    # the bg queues run dry. The ets pool (6 bufs) holds exactly two
    # rounds x 3 streams.
    prev = None  # (bi, kcs, {h: et})
    for bi, kcs in enumerate(batches):
        diag_phase = bi >= len(batches) - n_diag_batches
        if bi == len(batches) - n_diag_batches:
            while bg_must:
                bg_must.pop(0)()
        ebt = {}
        rot = [streams[(si + bi) % 3] for si in range(3)]
        for si, s in enumerate(rot):
            if si == 2:
                bg_slot(diag_phase)
            ebt[s["h"]] = do_scores(s, bi, kcs)
            if prev is not None:
                do_pv(s, prev[0], prev[1], prev[2][s["h"]])
                if w == TW - 1 and prev[0] == len(batches) - 2:
                    _normalize_half(nc, st, s, w, 0)
        prev = (bi, kcs, ebt)
    # flush: last batch's PV (+ normalize) for each stream. On the last
    # window the epilogue output projection is woven in per column half:
    # phase A (cols 0:256) runs as soon as the half-0 normalizes are done,
    # each psY-borrowing accumulator right after its stream's y retires.
    for si, s in enumerate(streams):
        if si == 2 and (bg_must or bg):
            (bg_must or bg).pop(0)()
        do_pv(s, prev[0], prev[1], prev[2][s["h"]])
        if w == TW - 1:
            _normalize_half(nc, st, s, w, 256)
        else:
            _normalize_a(nc, st, s, w)
    if w != TW - 1:
        st["pending_norm"] = [(s, w) for s in streams]

    # leftover background groups
    while bg:
        bg.pop(0)()


def _epi_start(nc, st, w):
    """Allocate the epilogue accumulators/staging: ec0/ec1 borrow psS,
    ec2 borrows psA, ec3..5 borrow the three psY slots (which free in
    stream flush order h0, h1, h2)."""
    pools = [(st["psS"], "s3"), (st["psS"], "s3"), (st["psA"], "acc"),
             (st["psY"], "y"), (st["psY"], "y"), (st["psY"], "y")]
    st["opst"] = [pool.tile([128, 512], F32, tag=tag, name=f"opse_{ec}")
                  for ec, (pool, tag) in enumerate(pools)]
    st["osb6e"] = st["outs"].tile([128, CC, 512], F16, tag="osb",
                                  name="osb_epi")


def _epi_a(nc, st, w, ecs):
    qs = w * 512
    for ec in ecs:
        nc.tensor.matmul(
            st["opst"][ec][:, 0:256],
            lhsT=st["wos01"][:, ec * 128:(ec + 1) * 128],
            rhs=st["ynA"][:, qs:qs + 256],
            start=True, stop=False, skip_group_check=True,
        )
        nc.tensor.matmul(
            st["opst"][ec][:, 0:256],
            lhsT=st["wos2"][:, ec * 128:(ec + 1) * 128],
            rhs=st["ynB"][:, qs:qs + 256],
            start=False, stop=True, skip_group_check=True,
        )


def _epi_evac(nc, st, w, c0):
    """evacuate + store one column half for all ec (Act copies: DVE is
    draining the normalize chains that gate these matmuls)."""
    qs = w * 512
    osb = st["osb6e"]
    for ec in range(CC):
        nc.scalar.copy(out=osb[:, ec, c0:c0 + 256],
                       in_=st["opst"][ec][:, c0:c0 + 256])
        if ec % 2 == 1:
            nc.sync.dma_start(
                out=st["outT"][128 * (ec - 1):128 * (ec + 1),
                               qs + c0:qs + c0 + 256].rearrange(
                    "(e p) c -> p e c", e=2),
                in_=osb[:, ec - 1:ec + 1, c0:c0 + 256],
            )


def _epi_b(nc, st, w):
    qs = w * 512
    for ec in range(CC):
        nc.tensor.matmul(
            st["opst"][ec][:, 256:512],
            lhsT=st["wos01"][:, ec * 128:(ec + 1) * 128],
            rhs=st["ynA"][:, qs + 256:qs + 512],
            start=True, stop=False, skip_group_check=True,
        )
        nc.tensor.matmul(
            st["opst"][ec][:, 256:512],
            lhsT=st["wos2"][:, ec * 128:(ec + 1) * 128],
            rhs=st["ynB"][:, qs + 256:qs + 512],
            start=False, stop=True, skip_group_check=True,
        )


def _normalize_half(nc, st, s, w, c0):
    """full normalize chain for one 256-wide column half (last window)."""
    qs = w * 512
    h = s["h"]
    y = s["y"]
    if c0 == 0:
        s["sc"] = st["scr"].tile([128, 512], F32, tag="sc",
                                 name=f"sc_h{h}_q{w}")
        s["rbt"] = st["scr"].tile([HD, 512], F32, tag="rbs",
                                  name=f"rb_h{h}_q{w}")
    sc, rb = s["sc"], s["rbt"]
    nc.vector.reciprocal(out=sc[0:1, c0:c0 + 256],
                         in_=y[HD:HD + 1, c0:c0 + 256])
    nc.gpsimd.partition_broadcast(rb[:, c0:c0 + 256], sc[0:1, c0:c0 + 256])
    nc.vector.tensor_mul(
        out=s["yn_ap"](qs + c0, 256),
        in0=y[0:HD, c0:c0 + 256], in1=rb[:, c0:c0 + 256],
    )


def _normalize_a(nc, st, s, w):
    """reciprocal of the sumexp row."""
    h = s["h"]
    y = s["y"]
    sc = st["scr"].tile([128, 512], F32, tag="sc", name=f"sc_h{h}_q{w}")
    s["sc"] = sc
    nc.vector.reciprocal(out=sc[0:1, 0:512], in_=y[HD:HD + 1, 0:512])


def _normalize_b(nc, st, s, w):
    """broadcast 1/sumexp to 64 partitions on GPSIMD, then
    y[0:64] * rb -> yn (Act copy + Pool multiply; see _normalize_half)."""
    qs = w * 512
    h = s["h"]
    y = s["y"]
    sc = s["sc"]
    rb = st["scr"].tile([HD, 512], F32, tag="rbs", name=f"rb_h{h}_q{w}")
    nc.gpsimd.partition_broadcast(rb[:, 0:512], sc[0:1, 0:512])
    nc.vector.tensor_mul(
        out=s["yn_ap"](qs, 512),
        in0=y[0:HD, 0:512], in1=rb[:, 0:512],
    )


def _prep_core_inputs(c, x, w_qkv, b_qkv, w_out):
    b = c // CPB
    g = c % CPB
    hs = [HPC * g + i for i in range(HPC)]

    def q8(a):
        return a.astype(E4NP)

    # fc column orders (interleaved for the d=2p+i split DMA):
    # fc0: [q_h0 d0, q_h1 d0, q_h0 d1, q_h1 d1, ...]
    # fc1: [q_h2 d0, k_h2 d0, q_h2 d1, k_h2 d1, ...]
    # fc2: [k_h0 d0, k_h1 d0, ...]
    d = np.arange(HD)
    qc = [h * HD + d for h in hs]                    # q cols per head
    kc_ = [D + h * HD + d for h in hs]               # k cols per head
    vc = [2 * D + h * HD + d for h in hs]

    fc0 = np.stack([qc[0], qc[1]], axis=1).reshape(-1)
    fc1 = np.stack([qc[2], kc_[2]], axis=1).reshape(-1)
    fc2 = np.stack([kc_[0], kc_[1]], axis=1).reshape(-1)
    cols = np.concatenate([fc0, fc1, fc2])
    vcols = np.concatenate(vc)

    wS = (w_qkv[:, cols] * WSC).astype(np.float32)
    wh = q8(wS)
    wvS = (w_qkv[:, vcols] * WSC).astype(np.float32)
    wvh_ = q8(wvS)
    wvl_ = q8(wvS - wvh_.astype(np.float32))

    xT = np.ascontiguousarray(x[b].T).astype(np.float32)
    xh_ = q8(xT)
    xl_ = q8(xT - xh_.astype(np.float32))

    # bias columns: fc0 = bq(h0|h1 interleaved)*WSC; fc1 = bq_h2 at even
    # partitions, 0 at odd (k_h2: bias dropped); fc2 = 0
    bq = b_qkv[:D]
    bcol = np.zeros((128, 3), dtype=np.float32)
    bcol[:, 0] = np.stack([bq[qc[0]], bq[qc[1]]], axis=1).reshape(-1) * WSC
    b1 = np.zeros(128, dtype=np.float32)
    b1[0::2] = bq[qc[2]] * WSC
    bcol[:, 1] = b1

    return {
        "xh": np.ascontiguousarray(xh_.reshape(CC, 128, T)),
        "xl": np.ascontiguousarray(xl_.reshape(CC, 128, T)),
        "wqh": np.ascontiguousarray(wh.reshape(CC, 128, 3, 128)),
        "bqk": bcol,
        "wvh": np.ascontiguousarray(wvh_.reshape(CC, 128, HPC * HD)),
        "wvl": np.ascontiguousarray(wvl_.reshape(CC, 128, HPC * HD)),
        "wo01": np.ascontiguousarray(
            w_out[192 * g:192 * g + 128, :].astype(np.float16)),
        "wo2": np.ascontiguousarray(
            w_out[192 * g + 128:192 * g + 192, :].astype(np.float16)),
    }


_NC_CACHE = {}


def get_nc():
    if "nc" not in _NC_CACHE:
        nc = build_bass()
        nc.finalize()
        _NC_CACHE["nc"] = nc
    return _NC_CACHE["nc"]


def kernel(x, w_qkv, b_qkv, w_out, b_out, _run_kwargs=None):
    x = np.asarray(x, dtype=np.float32)
    w_qkv = np.asarray(w_qkv, dtype=np.float32)
    b_qkv = np.asarray(b_qkv, dtype=np.float32)
    w_out = np.asarray(w_out, dtype=np.float32)
    b_out = np.asarray(b_out, dtype=np.float32)

    nc = get_nc()
    in_maps = [_prep_core_inputs(c, x, w_qkv, b_qkv, w_out)
               for c in range(NCORES)]
    kwargs = dict(_run_kwargs or {})
    res = run_bass_kernel_spmd(nc, in_maps, core_ids=list(range(NCORES)),
                               **kwargs)
    if kwargs:
        _NC_CACHE["last_results"] = res

    bv_corr = b_qkv[2 * D:3 * D] @ w_out  # [D]
    out = np.zeros((B, T, D), dtype=np.float32)
    for b in range(B):
        acc = np.zeros((T, D), dtype=np.float32)
        for g in range(CPB):
            acc += np.asarray(res.results[b * CPB + g]["outT"]
                              ).astype(np.float32).T
        out[b] = acc + (b_out + bv_corr)[None, :]
    return out


if __name__ == "__main__":
    nc = build_bass()
    print("built OK")


# revision 91
# speedup vs baseline: 1.2581x; 1.0199x over previous
"""Causal self-attention (B=2, T=2048, D=768, H=12) on 8 TRN2 NeuronCores.

Sharding: tensor-parallel over (batch, head) pairs; 3 heads per core, one
batch per 4-core group; the host sums the 4 partial outputs per batch and
adds b_out (+ the v-bias correction through w_out).

fp8 strategy (the PE cost model charges output-columns x cycles/row, with
fp8e4+DoubleRow at 0.5 cycles/row and contraction depth free):
- QKV / V projections run as 3-term residual fp8 DoubleRow matmuls:
  x = xh + xl (hi/lo fp8), w*64 = wh + wl, and
  psum = xh@wh + xh@wl + xl@wh (all same scale, lo*lo dropped, ~0.1% err).
  DoubleRow pairs adjacent contraction chunks ([128,2,*] APs), so the
  6-chunk contraction is 9 DR matmuls at half rate (vs 6 full-rate fp16).
- Scores run as fp8 DoubleRow with the head dim split 2x32: Q/K are
  evacuated to a flat fp8 tile (Q gets +bias*64; K's bias is dropped -- it
  is constant along the softmax axis, hence exactly softmax-invariant),
  then one SBUF->SBUF DMA per fc-group splits [128,512] into the
  [32,2,slot,512] layout (d = 2p+i interleave, heads pre-interleaved in
  the weight column order by the host).
- exp scale absorbs the 64*64 weight scaling: exp(s * SCALE/4096).
- PV and the output projection stay fp16 (attention-weight and value
  precision bound the error budget; measured ~1e-2 end-to-end).

Per 512-token q-window, the scores->exp->PV chain runs in "rounds"
of 2-k-chunk batches across the 3 head streams; below-diagonal chunks run
first, and background PE work (the window's K-side projection + V chunks,
the next window's Q-side projection, the previous window's output
projection, deferred normalize phases) is interleaved between rounds so
the PE keeps running while ScalarE exp latency drains. The softmax
normalizer uses a reciprocal on DVE plus a GPSIMD partition_broadcast.
A warm-up run of free N=1 matmuls starts the PE p-state ramp during the
initial DMA wait.

Constraints learned on real silicon (CoreSim does not model them):
GPSIMD cannot access PSUM at all; tensor_tensor cannot read two PSUM
operands; PSUM matmul outputs must start at partition 0.
"""

import numpy as np
import ml_dtypes

import concourse.bass as bass
import concourse.bacc as bacc
import concourse.mybir as mybir
import concourse.tile as tile
from concourse import library_config
from concourse.masks import make_upper_triangular
from concourse.bass_utils import run_bass_kernel_spmd

B, T, D, H, HD = 2, 2048, 768, 12, 64
NCORES = 8
HPC = 3            # heads per core
CPB = NCORES // B  # cores per batch = 4
CC = D // 128      # d_model chunks of 128 = 6
TW = T // 512      # token windows of 512 = 4
KC = T // 128      # k chunks of 128 = 16
SCALE = 1.0 / float(np.sqrt(HD))
WSC = 64.0         # fp8 weight pre-scale (folded into the exp scale / evacs)
ESCALE = SCALE / (WSC * WSC)   # exp scale for raw fp8-score psums

F8 = mybir.dt.float8e4
F16 = mybir.dt.float16
F32 = mybir.dt.float32
E4NP = ml_dtypes.float8_e4m3fn
DR = mybir.MatmulPerfMode.DoubleRow

EXP_BATCH = 2  # k-chunks per exp call / per s-tile (PSUM tile = 2 banks)

# QKD layout [32, fc, i, slot, T] (i = d-pair index, slot = head lane):
# per head h: q at (fcg, sl) and k at (fcg, sl) below
QPOS = ((0, 0), (0, 1), (1, 0))   # h0, h1, h2
KPOS = ((2, 0), (2, 1), (1, 1))

# Schraudolph fast-exp on DVE for stream h2's below-diagonal batches in
# the last window (relieves the ScalarE bottleneck there). exp(x) ~ bf16
# bitcast of int16(A*x + B); ~2.4% RMS approximation error on ~11% of the
# attention weights.
SCHRAU = True
SCHRAU_A = 128.0 / np.log(2.0)
SCHRAU_B = float(127 << 7) - 7.5


def build_bass():
    nc = bacc.Bacc(None, target_bir_lowering=False)

    xh = nc.dram_tensor("xh", [CC, 128, T], F8, kind="ExternalInput")
    xl = nc.dram_tensor("xl", [CC, 128, T], F8, kind="ExternalInput")
    wqh = nc.dram_tensor("wqh", [CC, 128, 3, 128], F8, kind="ExternalInput")
    bqk = nc.dram_tensor("bqk", [128, 3], F32, kind="ExternalInput")
    wvh = nc.dram_tensor("wvh", [CC, 128, HPC * HD], F8, kind="ExternalInput")
    wvl = nc.dram_tensor("wvl", [CC, 128, HPC * HD], F8, kind="ExternalInput")
    wo01d = nc.dram_tensor("wo01", [128, D], F16, kind="ExternalInput")
    wo2d = nc.dram_tensor("wo2", [HD, D], F16, kind="ExternalInput")
    outT = nc.dram_tensor("outT", [D, T], F16, kind="ExternalOutput")

    with tile.TileContext(nc) as tc:
        with (
            tc.tile_pool(name="big", bufs=1) as big,
            tc.tile_pool(name="ets", bufs=7) as ets,
            tc.tile_pool(name="scr", bufs=3) as scr,
            tc.tile_pool(name="outs", bufs=4) as outs,
            tc.tile_pool(name="psS", bufs=2, space="PSUM") as psS,
            tc.tile_pool(name="psY", bufs=3, space="PSUM") as psY,
            tc.tile_pool(name="psA", bufs=1, space="PSUM") as psA,
        ):
            # ---- SBUF persistent tiles ----
            wqhs = big.tile([128, CC, 3, 128], F8, tag="wqh")
            wvhs = big.tile([128, CC, HPC * HD], F8, tag="wvh")
            wvls = big.tile([128, CC, HPC * HD], F8, tag="wvl")
            xhs = big.tile([128, CC, T], F8, tag="xh")
            xls = big.tile([128, CC, T], F8, tag="xl")
            bqks = big.tile([128, 3], F32, tag="bqk")
            wos01 = big.tile([128, D], F16, tag="wo01")
            wos2 = big.tile([HD, D], F16, tag="wo2")
            QKF = big.tile([128, 3, T], F8, tag="QKF")      # flat staging
            QKD = big.tile([32, 3, 2, 2, T], F8, tag="QKD")  # DR split layout
            # token-major V (+ ones column at 64): [128, kc, h, 66]
            vT3 = big.tile([128, KC, HPC, 66], F16, tag="vT3")
            ynA = big.tile([128, T], F16, tag="ynA")
            ynB = big.tile([HD, T], F16, tag="ynB")
            mask_tri = big.tile([128, 128], F16, tag="mask")

            # PE p-state warm-up: a dense run of ~free N=1 matmuls starts
            # the tensor engine's ramp clock during the input-DMA wait so
            # the real matmuls reach full clock ~2us earlier; the dummy Exp
            # pulls the activation-table load (1.3us) off the first real
            # exp's critical path.
            wtiny = big.tile([1, 8], F16, tag="wtiny")
            nc.vector.memset(wtiny, 0.5)
            nc.scalar.activation(out=wtiny[0:1, 4:8], in_=wtiny[0:1, 0:4],
                                 func=mybir.ActivationFunctionType.Exp)
            s_warm = psS.tile([128, EXP_BATCH, 512], F32, tag="s3",
                              name="s_warm")
            warm_cols = 1024
            for i in range(120):
                j = i % warm_cols
                nc.tensor.matmul(
                    s_warm[0:1, j // 512, j % 512:j % 512 + 1],
                    lhsT=wtiny[0:1, 0:1], rhs=wtiny[0:1, 0:1],
                    start=True, stop=True, skip_group_check=True)

            # ---- input DMAs: window-0 criticals first. Weights go through
            # the Pool SWDGE queue while x window-0 chunks go through the
            # SP HWDGE queue -- two parallel descriptor-generation paths.
            nc.gpsimd.dma_start(
                out=wqhs[:, 0:2], in_=wqh[0:2].rearrange("c p f k -> p c f k"))
            nc.gpsimd.dma_start(out=xhs[:, 4, 0:512], in_=xh[4, :, 0:512])
            nc.gpsimd.dma_start(out=xhs[:, 5, 0:512], in_=xh[5, :, 0:512])
            nc.gpsimd.dma_start(
                out=wqhs[:, 4:6], in_=wqh[4:6].rearrange("c p f k -> p c f k"))
            # the GPSIMD ucode library carrying partition_broadcast loads
            # after the startup DMAs so it doesn't gate them
            nc.gpsimd.load_library(library_config.proxy)
            # SP queue: x hi window-0 + the q/k weight pair not on Pool
            nc.sync.dma_start(
                out=xhs[:, 0:2, 0:512],
                in_=xh[0:2, :, 0:512].rearrange("c p t -> p c t"))
            nc.sync.dma_start(
                out=wqhs[:, 2:4], in_=wqh[2:4].rearrange("c p f k -> p c f k"))
            nc.sync.dma_start(
                out=xhs[:, 2:4, 0:512],
                in_=xh[2:4, :, 0:512].rearrange("c p t -> p c t"))
            nc.sync.dma_start(out=bqks, in_=bqk[:, :])
            # Act queue (idle at startup): lo-residual inputs; V weights ride
            # the Pool SWDGE queue
            for c0 in (0, 2, 4):
                nc.scalar.dma_start(
                    out=xls[:, c0:c0 + 2, 0:512],
                    in_=xl[c0:c0 + 2, :, 0:512].rearrange("c p t -> p c t"))
            nc.gpsimd.dma_start(
                out=wvhs, in_=wvh.rearrange("c p f -> p c f"))
            nc.gpsimd.dma_start(
                out=wvls, in_=wvl.rearrange("c p f -> p c f"))
            # x tails: window-1 token range first so window-1 projections
            # (window-0 bg slots) aren't input-starved; single big DMAs to
            # keep the SP issue queue short
            nc.sync.dma_start(
                out=xhs[:, :, 512:1024],
                in_=xh[:, :, 512:1024].rearrange("c p t -> p c t"))
            nc.sync.dma_start(
                out=xls[:, :, 512:1024],
                in_=xl[:, :, 512:1024].rearrange("c p t -> p c t"))


            # ---- constants ----
            make_upper_triangular(nc, mask_tri, val=1.0, diag=True)
            for h in range(HPC):
                nc.gpsimd.memset(vT3[:, :, h, HD:HD + 1], 1.0)

            st = {
                "pending_norm": [],
                "wqhs": wqhs, "wvhs": wvhs, "wvls": wvls,
                "xhs": xhs, "xls": xls, "bqks": bqks,
                "wos01": wos01, "wos2": wos2, "QKF": QKF, "QKD": QKD,
                "vT3": vT3, "ynA": ynA, "ynB": ynB,
                "mask": mask_tri,
                "psS": psS, "psY": psY, "psA": psA,
                "ets": ets, "scr": scr, "outs": outs, "outT": outT,
            }

            # prologue: window-0 q/k projection. 3 psY accumulators, one per
            # fc group; term-major (hh, hl, lh) so each arriving input is
            # consumed immediately; V chunks 0/1 here, 2/3 ride window 0's
            # bg queue.
            paccs = [psY.tile([128, 512], F32, tag="y", name=f"pacc_f{fc}")
                     for fc in range(3)]
            # q/k: 2-term residual (x_hi + x_lo, single-fp8 weights); the
            # score-path tolerates the w-quantization since q/k are fp8
            # re-quantized at the evac anyway
            terms = [(wqhs, xhs), (wqhs, xls)]
            for ti, (wt, xt) in enumerate(terms):
                for c0 in (0, 2, 4):
                    for fc in range(3):
                        nc.tensor.matmul(
                            paccs[fc],
                            lhsT=wt[:, c0:c0 + 2, fc, :],
                            rhs=xt[:, c0:c0 + 2, 0:512],
                            start=(ti == 0 and c0 == 0),
                            stop=(ti == 1 and c0 == 4),
                            perf_mode=DR,
                        )
            # spread window-0's three split DMAs across three queues so
            # their descriptor generations run in parallel
            _evac_qk(nc, st, 0, 1, paccs[1], act=True, split=False)
            _evac_qk(nc, st, 0, 0, paccs[0], act=False, split=False)
            _evac_qk(nc, st, 0, 2, paccs[2], act=True, split=False)
            _split_qk(nc, st, 0, 1, queue=nc.gpsimd)
            _split_qk(nc, st, 0, 0, queue=nc.sync)
            _split_qk(nc, st, 0, 2, queue=nc.scalar)
            _proj_v_chunk(nc, st, 0)
            _proj_v_chunk(nc, st, 1, acc="s3")
            nc.gpsimd.dma_start(
                out=xhs[:, :, 1024:T],
                in_=xh[:, :, 1024:T].rearrange("c p t -> p c t"))
            nc.sync.dma_start(out=wos01, in_=wo01d[:, :])
            nc.sync.dma_start(out=wos2, in_=wo2d[:, :])
            nc.gpsimd.dma_start(
                out=xls[:, :, 1024:T],
                in_=xl[:, :, 1024:T].rearrange("c p t -> p c t"))

            for w in range(TW):
                _attn_window(nc, st, w)

            # epilogue: output projection for the last window
            qs = (TW - 1) * 512
            opst = []
            for ec in range(CC):
                pool, tag = [(psY, "y"), (psY, "y"), (psY, "y"),
                             (psS, "s3"), (psS, "s3"), (psA, "acc")][ec]
                opst.append(pool.tile([128, 512], F32, tag=tag,
                                      name=f"opse_{ec}"))
            osb6e = outs.tile([128, CC, 512], F16, tag="osb",
                              name="osb_epi")
            for ec in range(CC):
                for c0 in (0, 256):
                    nc.tensor.matmul(
                        opst[ec][:, c0:c0 + 256],
                        lhsT=wos01[:, ec * 128:(ec + 1) * 128],
                        rhs=ynA[:, qs + c0:qs + c0 + 256],
                        start=(c0 == 0), stop=False, skip_group_check=True,
                    )
                for c0 in (0, 256):
                    nc.tensor.matmul(
                        opst[ec][:, c0:c0 + 256],
                        lhsT=wos2[:, ec * 128:(ec + 1) * 128],
                        rhs=ynB[:, qs + c0:qs + c0 + 256],
                        start=False, stop=True, skip_group_check=True,
                    )
                if ec % 2 == 0:
                    nc.scalar.copy(out=osb6e[:, ec, :], in_=opst[ec])
                else:
                    nc.vector.tensor_copy(out=osb6e[:, ec, :],
                                          in_=opst[ec])
                if ec % 2 == 1:
                    q = nc.sync
                    q.dma_start(
                        out=outT[128 * (ec - 1):128 * (ec + 1),
                                 qs:qs + 512].rearrange(
                            "(e p) c -> p e c", e=2),
                        in_=osb6e[:, ec - 1:ec + 1, :],
                    )
    return nc


def _proj_qk_group(nc, st, w, fc, acc="acc"):
    """q/k projection for (window w, fc group): 9 DoubleRow matmuls
    (3 residual terms x 3 paired contraction chunks)."""
    ts_ = w * 512
    pool = st["psY"] if acc == "y" else st["psA"]
    ps = pool.tile([128, 512], F32, tag=acc, name=f"ps_f{fc}_t{w}")
    terms = [(st["wqhs"], st["xhs"]), (st["wqhs"], st["xls"])]
    for ti, (wt, xt) in enumerate(terms):
        for c0 in (0, 2, 4):
            nc.tensor.matmul(
                ps,
                lhsT=wt[:, c0:c0 + 2, fc, :],
                rhs=xt[:, c0:c0 + 2, ts_:ts_ + 512],
                start=(ti == 0 and c0 == 0),
                stop=(ti == 1 and c0 == 4),
                perf_mode=DR,
            )
    _evac_qk(nc, st, w, fc, ps)


def _evac_qk(nc, st, w, fc, ps, act=False, split=True):
    """Evacuate one fc group's psum to the flat fp8 tile (bias add for the
    q halves; the k bias columns are zero), then one SBUF->SBUF DMA to the
    [32,2,slot,512] DoubleRow layout (d = 2p+i interleave)."""
    ts_ = w * 512
    if act:
        nc.scalar.activation(
            out=st["QKF"][:, fc, ts_:ts_ + 512], in_=ps,
            func=mybir.ActivationFunctionType.Identity,
            bias=st["bqks"][:, fc:fc + 1])
    else:
        nc.vector.tensor_scalar(
            out=st["QKF"][:, fc, ts_:ts_ + 512], in0=ps,
            scalar1=st["bqks"][:, fc:fc + 1], scalar2=None,
            op0=mybir.AluOpType.add)
    if split:
        _split_qk(nc, st, w, fc)


def _split_qk(nc, st, w, fc, queue=None):
    ts_ = w * 512
    q = queue if queue is not None else (nc.gpsimd if w == 0 else nc.sync)
    q.dma_start(
        out=st["QKD"][:, fc, :, :, ts_:ts_ + 512],
        in_=st["QKF"][:, fc, ts_:ts_ + 512],
    )


def _proj_v_chunk(nc, st, tc_i, acc="acc"):
    """token-major V projection for one 128-token chunk: 9 DR matmuls."""
    pool = {"y": st["psY"], "s3": st["psS"],
            "acc": st["psA"]}[acc]
    psv = pool.tile([128, 512], F32, tag=acc, name=f"psv_{tc_i}")
    t0 = tc_i * 128
    terms = [(st["xhs"], st["wvhs"]), (st["xhs"], st["wvls"]),
             (st["xls"], st["wvhs"])]
    for ti, (xt, wt) in enumerate(terms):
        for c0 in (0, 2, 4):
            nc.tensor.matmul(
                psv[:, 0:HPC * HD],
                lhsT=xt[:, c0:c0 + 2, t0:t0 + 128],
                rhs=wt[:, c0:c0 + 2, :],
                start=(ti == 0 and c0 == 0),
                stop=(ti == 2 and c0 == 4),
                perf_mode=DR,
            )
    nc.vector.tensor_scalar(
        out=st["vT3"][:, tc_i, :, 0:HD],
        in0=psv[:, 0:HPC * HD].rearrange("p (h d) -> p h d", h=HPC),
        scalar1=1.0 / WSC, scalar2=None, op0=mybir.AluOpType.mult,
    )


def _outproj_group(nc, st, w, ec):
    qs = w * 512
    if ec == 0:
        st["osb6"] = st["outs"].tile([128, CC, 512], F16, tag="osb",
                                     name=f"osb_q{w}")
    ops = st["psA"].tile([128, 512], F32, tag="acc", name=f"ops_e{ec}_q{w}")
    nc.tensor.matmul(
        ops,
        lhsT=st["wos01"][:, ec * 128:(ec + 1) * 128],
        rhs=st["ynA"][:, qs:qs + 512],
        start=True, stop=False,
    )
    nc.tensor.matmul(
        ops,
        lhsT=st["wos2"][:, ec * 128:(ec + 1) * 128],
        rhs=st["ynB"][:, qs:qs + 512],
        start=False, stop=True,
    )
    nc.vector.tensor_copy(out=st["osb6"][:, ec, :], in_=ops)
    if ec == CC - 1:
        nc.sync.dma_start(
            out=st["outT"][:, qs:qs + 512].rearrange(
                "(e p) c -> p e c", e=CC),
            in_=st["osb6"],
        )


def _attn_window(nc, st, w):
    """Attention for q-window w across the 3 head streams, with background
    PE work (next-window projection, previous-window outproj) interleaved
    between score/PV rounds."""
    qs = w * 512
    nchunks = 4 * (w + 1)
    # below-diagonal chunks first (descending, so the first PV write is the
    # full column range), diagonal chunks last: the window's own K-side
    # projection (fc2) and V chunks then slide into this window's early
    # rounds instead of crowding the previous one.
    kc_order = list(range(4 * w))[::-1] + list(range(4 * w, nchunks))
    batches = [kc_order[i:i + EXP_BATCH]
               for i in range(0, nchunks, EXP_BATCH)]
    n_diag_batches = 2

    # bg_must: work that must land before this window's diagonal rounds.
    # bg_opt: deferrable work (previous window's normalize phase B and
    # outproj, next window's Q-side projection).
    bg_must = []
    if w == 0:
        for j in (2, 3):
            bg_must.append(lambda j=j: _proj_v_chunk(nc, st, j))
    else:
        bg_must.append(lambda: _proj_qk_group(nc, st, w, 2))
        for j in range(4):
            bg_must.append(lambda j=j: _proj_v_chunk(nc, st, 4 * w + j))
    # issue the previous window's normalize phase B immediately: it only
    # uses DVE/Pool, and it must release the y-PSUM slots before this
    # window's first (pipelined) PV round claims them.
    for s_, w_ in st.pop("pending_norm", []):
        _normalize_b(nc, st, s_, w_)
    bg_opt = []
    if w + 1 < TW:
        for fc in (0, 1):
            bg_opt.append(lambda fc=fc: _proj_qk_group(nc, st, w + 1, fc))
    if w >= 1:
        for ec in range(CC):
            bg_opt.append(lambda ec=ec: _outproj_group(nc, st, w - 1, ec))

    def bg_slot(diag_phase=False):
        if bg_must:
            bg_must.pop(0)()
        elif bg_opt:
            # hold back two opt groups to feed the PE during the
            # Act-bound diagonal rounds
            if diag_phase or len(bg_opt) > 2 or w == 0:
                bg_opt.pop(0)()

    bg = bg_opt  # leftover drain at window end uses the opt queue

    streams = [
        {"h": 0, "yn_ap": lambda q, n: st["ynA"][0:HD, q:q + n]},
        {"h": 1, "yn_ap": lambda q, n: st["ynA"][HD:128, q:q + n]},
        {"h": 2, "yn_ap": lambda q, n: st["ynB"][0:HD, q:q + n]},
    ]
    if w == 0:
        # fc1 (h2's q/k) is evacuated+split first in the prologue
        streams = [streams[2], streams[0], streams[1]]
    for s in streams:
        s["y"] = st["psY"].tile([128, 512], F32, tag="y",
                                name=f"y_h{s['h']}_q{w}")

    QKD = st["QKD"]

    def do_scores(s, bi, kcs):
        """scores + exp for one stream/batch; returns the et tile."""
        nb = len(kcs)
        h = s["h"]
        schrau = (SCHRAU and w == TW - 1 and h == 2
                  and all(kc < 4 * w for kc in kcs)
                  and bi >= 4)
        s_ps = st["psS"].tile([128, EXP_BATCH, 512], F32, tag="s3",
                              name=f"s_h{h}_q{w}_b{bi}")
        if schrau:
            eti = st["ets"].tile([128, EXP_BATCH, 512], mybir.dt.int16,
                                 tag="et", name=f"e_h{h}_q{w}_b{bi}")
            et = eti.bitcast(mybir.dt.bfloat16)
        else:
            et = st["ets"].tile([128, EXP_BATCH, 512], F16, tag="et",
                                name=f"e_h{h}_q{w}_b{bi}")
        js = [max(0, kc - 4 * w) for kc in kcs]
        jw = [min(js)] * nb
        kf, ks = KPOS[h]
        qf, qsl = QPOS[h]
        for i in range(nb):
            kc = kcs[i]
            j = jw[i]
            nc.tensor.matmul(
                s_ps[:, i, 128 * j:512],
                lhsT=QKD[:, kf, :, ks, kc * 128:(kc + 1) * 128],
                rhs=QKD[:, qf, :, qsl, qs + 128 * j:qs + 512],
                start=True, stop=True,
                perf_mode=DR,
            )
        if schrau:
            nc.vector.tensor_scalar(
                out=eti[:, 0:nb, :], in0=s_ps[:, 0:nb, :],
                scalar1=SCHRAU_A * ESCALE, scalar2=SCHRAU_B,
                op0=mybir.AluOpType.mult, op1=mybir.AluOpType.add)
            return et
        jm = min(js)
        nc.scalar.activation(
            out=et[:, 0:nb, 128 * jm:512],
            in_=s_ps[:, 0:nb, 128 * jm:512],
            func=mybir.ActivationFunctionType.Exp, scale=ESCALE,
        )
        for i in range(nb):
            j = kcs[i] - 4 * w
            if j < 0:
                continue
            nc.gpsimd.tensor_mul(
                out=et[:, i, 128 * j:128 * (j + 1)],
                in0=et[:, i, 128 * j:128 * (j + 1)],
                in1=st["mask"],
            )
        return et

    def do_pv(s, bi, kcs, et):
        for i in range(len(kcs)):
            kc = kcs[i]
            j = max(0, kc - 4 * w)
            idx = bi * EXP_BATCH + i
            nc.tensor.matmul(
                s["y"][0:HD + 1, 128 * j:512],
                lhsT=st["vT3"][:, kc, s["h"], 0:HD + 1],
                rhs=et[:, i, 128 * j:512],
                start=(idx == 0), stop=(idx == nchunks - 1),
                skip_group_check=True,
            )

    # software-pipelined rounds: batch b's scores/exp run interleaved with
    # batch b-1's PV, so the PE never sits behind an exp chain even when
    # the bg queues run dry. The ets pool (6 bufs) holds exactly two
    # rounds x 3 streams.
    prev = None  # (bi, kcs, {h: et})
    for bi, kcs in enumerate(batches):
        diag_phase = bi >= len(batches) - n_diag_batches
        if bi == len(batches) - n_diag_batches:
            while bg_must:
                bg_must.pop(0)()
        ebt = {}
        rot = [streams[(si + bi) % 3] for si in range(3)]
        for si, s in enumerate(rot):
            if si == 2:
                bg_slot(diag_phase)
            ebt[s["h"]] = do_scores(s, bi, kcs)
            if prev is not None:
                do_pv(s, prev[0], prev[1], prev[2][s["h"]])
                if prev[0] == len(batches) - 2:
                    if w == TW - 1:
                        _normalize_half(nc, st, s, w, 0)
                    else:
                        _normalize_a(nc, st, s, w, 0)
        prev = (bi, kcs, ebt)
    # flush: last batch's PV (+ normalize) for each stream. On the last
    # window the epilogue output projection is woven in per column half:
    # phase A (cols 0:256) runs as soon as the half-0 normalizes are done,
    # each psY-borrowing accumulator right after its stream's y retires.
    for si, s in enumerate(streams):
        if si == 2 and (bg_must or bg):
            (bg_must or bg).pop(0)()
        do_pv(s, prev[0], prev[1], prev[2][s["h"]])
        if w == TW - 1:
            _normalize_half(nc, st, s, w, 256)
        else:
            _normalize_a(nc, st, s, w, 256)
    if w != TW - 1:
        st["pending_norm"] = [(s, w) for s in streams]

    # leftover background groups
    while bg:
        bg.pop(0)()


def _epi_start(nc, st, w):
    """Allocate the epilogue accumulators/staging: ec0/ec1 borrow psS,
    ec2 borrows psA, ec3..5 borrow the three psY slots (which free in
    stream flush order h0, h1, h2)."""
    pools = [(st["psS"], "s3"), (st["psS"], "s3"), (st["psA"], "acc"),
             (st["psY"], "y"), (st["psY"], "y"), (st["psY"], "y")]
    st["opst"] = [pool.tile([128, 512], F32, tag=tag, name=f"opse_{ec}")
                  for ec, (pool, tag) in enumerate(pools)]
    st["osb6e"] = st["outs"].tile([128, CC, 512], F16, tag="osb",
                                  name="osb_epi")


def _epi_a(nc, st, w, ecs):
    qs = w * 512
    for ec in ecs:
        nc.tensor.matmul(
            st["opst"][ec][:, 0:256],
            lhsT=st["wos01"][:, ec * 128:(ec + 1) * 128],
            rhs=st["ynA"][:, qs:qs + 256],
            start=True, stop=False, skip_group_check=True,
        )
        nc.tensor.matmul(
            st["opst"][ec][:, 0:256],
            lhsT=st["wos2"][:, ec * 128:(ec + 1) * 128],
            rhs=st["ynB"][:, qs:qs + 256],
            start=False, stop=True, skip_group_check=True,
        )


def _epi_evac(nc, st, w, c0):
    """evacuate + store one column half for all ec (Act copies: DVE is
    draining the normalize chains that gate these matmuls)."""
    qs = w * 512
    osb = st["osb6e"]
    for ec in range(CC):
        nc.scalar.copy(out=osb[:, ec, c0:c0 + 256],
                       in_=st["opst"][ec][:, c0:c0 + 256])
        if ec % 2 == 1:
            nc.sync.dma_start(
                out=st["outT"][128 * (ec - 1):128 * (ec + 1),
                               qs + c0:qs + c0 + 256].rearrange(
                    "(e p) c -> p e c", e=2),
                in_=osb[:, ec - 1:ec + 1, c0:c0 + 256],
            )


def _epi_b(nc, st, w):
    qs = w * 512
    for ec in range(CC):
        nc.tensor.matmul(
            st["opst"][ec][:, 256:512],
            lhsT=st["wos01"][:, ec * 128:(ec + 1) * 128],
            rhs=st["ynA"][:, qs + 256:qs + 512],
            start=True, stop=False, skip_group_check=True,
        )
        nc.tensor.matmul(
            st["opst"][ec][:, 256:512],
            lhsT=st["wos2"][:, ec * 128:(ec + 1) * 128],
            rhs=st["ynB"][:, qs + 256:qs + 512],
            start=False, stop=True, skip_group_check=True,
        )


def _normalize_half(nc, st, s, w, c0):
    """full normalize chain for one 256-wide column half (last window)."""
    qs = w * 512
    h = s["h"]
    y = s["y"]
    if c0 == 0:
        s["sc"] = st["scr"].tile([128, 512], F32, tag="sc",
                                 name=f"sc_h{h}_q{w}")
        s["rbt"] = st["scr"].tile([HD, 512], F32, tag="rbs",
                                  name=f"rb_h{h}_q{w}")
    if c0 == 0:
        s["yct"] = st["scr"].tile([HD, 512], F32, tag="yc",
                                  name=f"yc_h{h}_q{w}")
    sc, rb, yc = s["sc"], s["rbt"], s["yct"]
    nc.vector.reciprocal(out=sc[0:1, c0:c0 + 256],
                         in_=y[HD:HD + 1, c0:c0 + 256])
    nc.gpsimd.partition_broadcast(rb[:, c0:c0 + 256], sc[0:1, c0:c0 + 256])
    # last-window normalize: y leaves PSUM via an Act copy and the multiply
    # runs on Pool -- Act/Pool idle at the tail while DVE is the jam
    nc.scalar.copy(out=yc[:, c0:c0 + 256], in_=y[0:HD, c0:c0 + 256])
    nc.gpsimd.tensor_mul(
        out=s["yn_ap"](qs + c0, 256),
        in0=yc[:, c0:c0 + 256], in1=rb[:, c0:c0 + 256],
    )


def _normalize_a(nc, st, s, w, c0):
    """reciprocal of the sumexp row, one column half at a time (the diag
    chunks finalize sumexp[0:256] a round early)."""
    h = s["h"]
    if c0 == 0:
        s["sc"] = st["scr"].tile([128, 512], F32, tag="sc",
                                 name=f"sc_h{h}_q{w}")
    nc.vector.reciprocal(out=s["sc"][0:1, c0:c0 + 256],
                         in_=s["y"][HD:HD + 1, c0:c0 + 256])


def _normalize_b(nc, st, s, w):
    """broadcast 1/sumexp to 64 partitions on GPSIMD, then
    y[0:64] * rb -> yn (Act copy + Pool multiply; see _normalize_half)."""
    qs = w * 512
    h = s["h"]
    y = s["y"]
    sc = s["sc"]
    rb = st["scr"].tile([HD, 512], F32, tag="rbs", name=f"rb_h{h}_q{w}")
    nc.gpsimd.partition_broadcast(rb[:, 0:512], sc[0:1, 0:512])
    nc.vector.tensor_mul(
        out=s["yn_ap"](qs, 512),
        in0=y[0:HD, 0:512], in1=rb[:, 0:512],
    )


def _prep_core_inputs(c, x, w_qkv, b_qkv, w_out):
    b = c // CPB
    g = c % CPB
    hs = [HPC * g + i for i in range(HPC)]

    def q8(a):
        return a.astype(E4NP)

    # fc column orders (interleaved for the d=2p+i split DMA):
    # fc0: [q_h0 d0, q_h1 d0, q_h0 d1, q_h1 d1, ...]
    # fc1: [q_h2 d0, k_h2 d0, q_h2 d1, k_h2 d1, ...]
    # fc2: [k_h0 d0, k_h1 d0, ...]
    d = np.arange(HD)
    qc = [h * HD + d for h in hs]                    # q cols per head
    kc_ = [D + h * HD + d for h in hs]               # k cols per head
    vc = [2 * D + h * HD + d for h in hs]

    fc0 = np.stack([qc[0], qc[1]], axis=1).reshape(-1)
    fc1 = np.stack([qc[2], kc_[2]], axis=1).reshape(-1)
    fc2 = np.stack([kc_[0], kc_[1]], axis=1).reshape(-1)
    cols = np.concatenate([fc0, fc1, fc2])
    vcols = np.concatenate(vc)

    wS = (w_qkv[:, cols] * WSC).astype(np.float32)
    wh = q8(wS)
    wvS = (w_qkv[:, vcols] * WSC).astype(np.float32)
    wvh_ = q8(wvS)
    wvl_ = q8(wvS - wvh_.astype(np.float32))

    xT = np.ascontiguousarray(x[b].T).astype(np.float32)
    xh_ = q8(xT)
    xl_ = q8(xT - xh_.astype(np.float32))

    # bias columns: fc0 = bq(h0|h1 interleaved)*WSC; fc1 = bq_h2 at even
    # partitions, 0 at odd (k_h2: bias dropped); fc2 = 0
    bq = b_qkv[:D]
    bcol = np.zeros((128, 3), dtype=np.float32)
    bcol[:, 0] = np.stack([bq[qc[0]], bq[qc[1]]], axis=1).reshape(-1) * WSC
    b1 = np.zeros(128, dtype=np.float32)
    b1[0::2] = bq[qc[2]] * WSC
    bcol[:, 1] = b1

    return {
        "xh": np.ascontiguousarray(xh_.reshape(CC, 128, T)),
        "xl": np.ascontiguousarray(xl_.reshape(CC, 128, T)),
        "wqh": np.ascontiguousarray(wh.reshape(CC, 128, 3, 128)),
        "bqk": bcol,
        "wvh": np.ascontiguousarray(wvh_.reshape(CC, 128, HPC * HD)),
        "wvl": np.ascontiguousarray(wvl_.reshape(CC, 128, HPC * HD)),
        "wo01": np.ascontiguousarray(
            w_out[192 * g:192 * g + 128, :].astype(np.float16)),
        "wo2": np.ascontiguousarray(
            w_out[192 * g + 128:192 * g + 192, :].astype(np.float16)),
    }


_NC_CACHE = {}


def get_nc():
    if "nc" not in _NC_CACHE:
        nc = build_bass()
        nc.finalize()
        _NC_CACHE["nc"] = nc
    return _NC_CACHE["nc"]


def kernel(x, w_qkv, b_qkv, w_out, b_out, _run_kwargs=None):
    x = np.asarray(x, dtype=np.float32)
    w_qkv = np.asarray(w_qkv, dtype=np.float32)
    b_qkv = np.asarray(b_qkv, dtype=np.float32)
    w_out = np.asarray(w_out, dtype=np.float32)
    b_out = np.asarray(b_out, dtype=np.float32)

    nc = get_nc()
    in_maps = [_prep_core_inputs(c, x, w_qkv, b_qkv, w_out)
               for c in range(NCORES)]
    kwargs = dict(_run_kwargs or {})
    res = run_bass_kernel_spmd(nc, in_maps, core_ids=list(range(NCORES)),
                               **kwargs)
    if kwargs:
        _NC_CACHE["last_results"] = res

    bv_corr = b_qkv[2 * D:3 * D] @ w_out  # [D]
    out = np.zeros((B, T, D), dtype=np.float32)
    for b in range(B):
        acc = np.zeros((T, D), dtype=np.float32)
        for g in range(CPB):
            acc += np.asarray(res.results[b * CPB + g]["outT"]
                              ).astype(np.float32).T
        out[b] = acc + (b_out + bv_corr)[None, :]
    return out


if __name__ == "__main__":
    nc = build_bass()
    print("built OK")
